# revision 43
# baseline (speedup 1.0000x reference)
"""Trainium2 Bass kernel for nn_MCFL_49254684950998 (dense multimodal transformer block).

Strategy: pure data parallel over 8 NeuronCores (batch 16384 -> 2048/core).
Feature-major layout ("T layout": [feat_chunk=128, batch]); host pre-transposes
inputs and post-transposes outputs. All GEMMs run in bf16 (full PE rate,
fp32 PSUM accumulate). Attention (3-token self-attn + 2-token cross-attn)
uses DVE bf16 elementwise products + selection matmuls on the TensorEngine;
softmax and LayerNorm reciprocals via ACT ln/exp (one act table, no swaps);
LayerNorm stats via col-packed ones-matmuls, apply via rank-1 PE broadcasts
plus fused tensor_scalar. Buffers are pool-allocated with bufs>=2 on the
block-critical paths so the Tile scheduler overlaps adjacent batch blocks
(keeps PE warm / HAM un-throttled).
"""

import sys

sys.path.insert(0, "/opt/trn_rl_repo")

import numpy as np
import ml_dtypes

import concourse.bass as bass
import concourse.bacc as bacc
import concourse.tile as tile
import concourse.mybir as mybir
from concourse import bass_utils

F32 = mybir.dt.float32
BF16 = mybir.dt.bfloat16
AF = mybir.ActivationFunctionType
OP = mybir.AluOpType

B, D, H, HD = 16384, 1024, 16, 64
NCORES = 8
BLOC = B // NCORES          # 2048 batch rows per core
BF = 512                    # batch tile (free dim) per block
NBLK_HW = BLOC // BF        # 4 blocks per core
NCH = D // 128              # 8 feature chunks
SCALE = HD ** -0.5
EPS = 1e-5


def build(tc, outs, ins, nblk):
    from contextlib import ExitStack
    stack = ExitStack()
    nc = tc.nc
    out_t = outs["out_t"]
    xt = [ins["xt_t"], ins["xt_i"], ins["xt_a"]]

    # ---- const tiles (loaded once) ----
    cpool = stack.enter_context(tc.tile_pool(name="consts", bufs=1))
    sel_sb = cpool.tile([128, NCH * 16], BF16, tag="sel")       # [128, c, 16]
    nc.sync.dma_start(sel_sb[:], ins["sel"][:])
    selb_sb = cpool.tile([96, NCH * 128], BF16, tag="selb")     # bases 0/32/64
    nc.sync.dma_start(selb_sb[:], ins["selb"][:])
    ones_sb = cpool.tile([128, 1], BF16, tag="ones")            # 1/1024
    nc.sync.dma_start(ones_sb[:], ins["ones_col"][:])
    onesrow_sb = cpool.tile([65, 128], BF16, tag="onesrow")     # 1.0 @ rows 0/32/64
    nc.sync.dma_start(onesrow_sb[:], ins["ones_row"][:])
    cols = {}
    for nm in ("sab", "l1g", "l1b", "cab", "l2g", "l2b"):
        cols[nm] = cpool.tile([128, NCH], F32, tag=nm, name=f"col_{nm}")
        nc.sync.dma_start(cols[nm][:], ins[nm][:])

    def sel_c(c):
        return sel_sb[:, c * 16:(c + 1) * 16]

    def selb_c(c, base=0):
        return selb_sb[base:base + 16, c * 128:(c + 1) * 128]

    # ---- pools ----
    def pool(*a, **k):
        return stack.enter_context(tc.tile_pool(*a, **k))

    wq_pool = pool(name="wqkv", bufs=3)       # [128, 8*384] bf16 (6KB)
    wog_pool = pool(name="wog", bufs=3)       # [128, 8*256] bf16 (4KB)
    x_pool = pool(name="xs", bufs=2)          # 24 tags [128,BF] bf16
    qk_pool = pool(name="qk", bufs=8)         # [128, BF] bf16
    v_pool = pool(name="vs", bufs=1)          # 24 tags [128,BF] bf16
    pr_pool = pool(name="prod", bufs=4)       # [128, BF] bf16
    tt_pool = pool(name="tt", bufs=4)         # [128, BF] bf16 (attnout terms)
    ao_pool = pool(name="ao", bufs=2)         # 8 tags [128,BF] bf16
    cq_pool = pool(name="cq", bufs=1)         # 8 tags [128, BF] bf16
    ck_pool = pool(name="ck", bufs=4)         # [128, BF] bf16
    cv_pool = pool(name="cv", bufs=1)         # 16 tags [128,BF] bf16
    sq_pool = pool(name="sq", bufs=2)         # [128, BF] bf16 squares
    sm_pool = pool(name="sm", bufs=1)         # small f32/bf16 softmax+LN tiles
    pp_pool = pool(name="pp", bufs=2)         # softmax P tiles (cross-block)
    out_pool = pool(name="outp", bufs=1)      # 2 tags [128,BF] f32
    rbc_pool = pool(name="rbc", bufs=2)       # [128,BF] bf16 rstd broadcast
    ps_big = pool(name="psbig", bufs=5, space="PSUM")
    ps_S = pool(name="psS", bufs=3, space="PSUM")

    def ln_group(ylists, gcol, bcol, dstlists):
        """Layernorm over the feature (partition-chunk) dim for 1-3 tokens at
        once. ylists/dstlists: per-token lists of 8 [128,BF] APs. Token t's
        stats live at partition 32t of two PSUM banks (col-group packing);
        smalls processed on rows [0:R] in one shot (junk rows harmless)."""
        ntok = len(ylists)
        R = 32 * (ntok - 1) + 1
        stA = ps_S.tile([128, BF], F32, tag="S", name="stA")
        stB = ps_S.tile([128, BF], F32, tag="S", name="stB")
        # all squares first (DVE/GP split) so the stats matmuls never starve
        sqs = {}
        for t in range(ntok):
            for c in range(NCH):
                sq = sq_pool.tile([128, BF], BF16, tag="sq")
                eng = nc.gpsimd if (t * NCH + c) % 3 == 0 else nc.vector
                eng.tensor_tensor(sq[:], ylists[t][c], ylists[t][c], op=OP.mult)
                sqs[(t, c)] = sq
        for t in range(ntok):
            b0 = 32 * t
            for c in range(NCH):
                nc.tensor.matmul(stA[b0:b0 + 1, :], ones_sb[:], ylists[t][c],
                                 start=(c == 0), stop=(c == NCH - 1),
                                 tile_position=(0, b0), skip_group_check=True)
            for c in range(NCH):
                nc.tensor.matmul(stB[b0:b0 + 1, :], ones_sb[:], sqs[(t, c)][:],
                                 start=(c == 0), stop=(c == NCH - 1),
                                 tile_position=(0, b0), skip_group_check=True)
        mu_s = sm_pool.tile([65, BF], F32, tag="mu_s")
        nc.vector.tensor_copy(mu_s[0:R, :], stA[0:R, :])
        mu2 = sm_pool.tile([65, BF], F32, tag="lnvr", name="mu2")
        nc.scalar.activation(mu2[0:R, :], stA[0:R, :], AF.Square)
        var = sm_pool.tile([65, BF], F32, tag="var")
        nc.vector.scalar_tensor_tensor(var[0:R, :], stB[0:R, :], EPS, mu2[0:R, :],
                                       op0=OP.add, op1=OP.subtract)
        # keep-warm matmul chained on var: bridges the smalls serial chain
        nc.tensor.matmul(stA[96:112, 0:64], sel_sb[0:R, 0:16],
                         var[:].bitcast(BF16)[0:R, 0:64], start=True, stop=True,
                         tile_position=(0, 96), skip_group_check=True)
        lnv = sm_pool.tile([65, BF], F32, tag="lnvr")
        nc.scalar.activation(lnv[0:R, :], var[0:R, :], AF.Ln)
        rstd = sm_pool.tile([65, BF], BF16, tag="rstd")
        nc.scalar.activation(rstd[0:R, :], lnv[0:R, :], AF.Exp, scale=-0.5)
        mup = sm_pool.tile([65, BF], BF16, tag="mup")
        nc.vector.tensor_tensor(mup[0:R, :], mu_s[0:R, :], rstd[0:R, :], op=OP.mult)
        for t in range(ntok):
            b0 = 32 * t
            rb_ps = ps_S.tile([128, BF], F32, tag="S", name="rb_ps")
            nc.tensor.matmul(rb_ps[:], onesrow_sb[b0:b0 + 1, :],
                             rstd[b0:b0 + 1, :], start=True, stop=True)
            rb = rbc_pool.tile([128, BF], BF16, tag="rbc")
            nc.scalar.copy(rb[:], rb_ps[:])
            mu_ps = ps_S.tile([128, BF], F32, tag="S", name="mu_ps")
            nc.tensor.matmul(mu_ps[:], onesrow_sb[b0:b0 + 1, :],
                             mup[b0:b0 + 1, :], start=True, stop=True)
            for c in range(NCH):
                t1 = tt_pool.tile([128, BF], BF16, tag="tt")
                nc.vector.tensor_tensor(t1[:], ylists[t][c], rb[:], op=OP.mult)
                t2 = tt_pool.tile([128, BF], BF16, tag="tt")
                nc.vector.tensor_tensor(t2[:], t1[:], mu_ps[:], op=OP.subtract)
                nc.vector.tensor_scalar(dstlists[t][c], t2[:], gcol[:, c:c + 1],
                                        bcol[:, c:c + 1], op0=OP.mult, op1=OP.add)

    def head(blk):
        """Phases 1-3: x load, qkv GEMM + scores, softmax -> P tiles."""
        bs = blk * BF
        # first weight slab before the x queue so the PE isn't DMA-gated
        wt0 = wq_pool.tile([128, NCH * 384], BF16, tag="wqkv", name="wt0")
        wsl0 = ins["wqkv_p"][:, 0:NCH * 384]
        half = NCH * 192
        nc.sync.dma_start(wt0[:, :half], wsl0[:, :half])
        nc.scalar.dma_start(wt0[:, half:], wsl0[:, half:])
        xs = {}
        qs = (nc.sync, nc.scalar)
        for m in range(3):
            for c in range(NCH):
                t = x_pool.tile([128, BF], BF16, tag=f"x_{m}_{c}")
                qs[(m * NCH + c) % 2].dma_start(
                    t[:], xt[m][c * 128:(c + 1) * 128, bs:bs + BF])
                xs[(m, c)] = t

        S_banks = [ps_S.tile([128, BF], F32, tag="S", name=f"Sbank{i}")
                   for i in range(3)]
        vs = {}

        def emit_products(c, qts, kts):
            for i in range(3):
                for j in range(3):
                    pr = pr_pool.tile([128, BF], BF16, tag="prod")
                    nc.vector.tensor_tensor(pr[:], qts[i][:], kts[j][:], op=OP.mult)
                    nc.tensor.matmul(
                        S_banks[j][32 * i:32 * i + 16, :],
                        sel_c(c), pr[:],
                        start=(c == 0), stop=(c == NCH - 1),
                        tile_position=(0, 32 * i),
                        skip_group_check=True,
                    )

        for c in range(NCH):
            if c == 0:
                wt = wt0
            else:
                wt = wq_pool.tile([128, NCH * 384], BF16, tag="wqkv")
                wsl = ins["wqkv_p"][:, c * NCH * 384:(c + 1) * NCH * 384]
                nc.sync.dma_start(wt[:, :half], wsl[:, :half])
                nc.scalar.dma_start(wt[:, half:], wsl[:, half:])
            qts, kts = [], []
            for m in range(3):
                psq = ps_big.tile([128, BF], F32, tag="big")
                psk = ps_big.tile([128, BF], F32, tag="big")
                psv = ps_big.tile([128, BF], F32, tag="big")
                for k in range(NCH):
                    wk = wt[:, k * 384:(k + 1) * 384]
                    st, sp = (k == 0), (k == NCH - 1)
                    nc.tensor.matmul(psq[:], wk[:, 0:128], xs[(m, k)][:], start=st, stop=sp)
                    nc.tensor.matmul(psk[:], wk[:, 128:256], xs[(m, k)][:], start=st, stop=sp)
                    nc.tensor.matmul(psv[:], wk[:, 256:384], xs[(m, k)][:], start=st, stop=sp)
                qt = qk_pool.tile([128, BF], BF16, tag="qk")
                kt = qk_pool.tile([128, BF], BF16, tag="qk")
                vt = v_pool.tile([128, BF], BF16, tag=f"v_{m}_{c}")
                nc.scalar.copy(qt[:], psq[:])
                nc.scalar.copy(kt[:], psk[:])
                nc.scalar.copy(vt[:], psv[:])
                qts.append(qt)
                kts.append(kt)
                vs[(m, c)] = vt
            emit_products(c, qts, kts)

        def warm(src):
            """Tiny matmul chained on `src` ([80,BF] tile) into unused rows of
            S bank 0 — keeps the PE HAM window busy through the softmax
            serial chain (a >3.4us PE-idle gap would re-throttle the clock).
            Values are garbage (bitcast) — the output rows are never read."""
            mv = src[:].bitcast(BF16) if src.dtype == F32 else src[:]
            nc.tensor.matmul(S_banks[0][96:112, 0:64], sel_sb[0:80, 0:16],
                             mv[0:80, 0:64], start=True, stop=True,
                             tile_position=(0, 96), skip_group_check=True)

        # softmax over 3 keys; tokens packed at partition bases 0/32/64
        E = []
        for j in range(3):
            e = sm_pool.tile([80, BF], F32, tag=f"E{j}", name=f"E{j}")
            nc.scalar.activation(e[:], S_banks[j][0:80, :], AF.Exp)
            E.append(e)
        warm(E[0])
        esum = sm_pool.tile([80, BF], F32, tag="esum")
        nc.vector.tensor_tensor(esum[:], E[0][:], E[1][:], op=OP.add)
        nc.vector.tensor_tensor(esum[:], esum[:], E[2][:], op=OP.add)
        warm(esum)
        rec = sm_pool.tile([80, BF], F32, tag="rec")
        nc.vector.reciprocal_approx_fast(rec[:], esum[:])
        warm(rec)
        P = []
        for j in range(3):
            p = pp_pool.tile([80, BF], BF16, tag=f"P{j}", name=f"P{j}")
            nc.vector.tensor_tensor(p[:], E[j][:], rec[:], op=OP.mult)
            P.append(p)
        return dict(bs=bs, xs=xs, vs=vs, P=P)

    def tail(st):
        """Phases 4-7 for a block whose head already ran."""
        bs, xs, vs, P = st["bs"], st["xs"], st["vs"], st["P"]
        # ---- phase 4: per token: weighted sum, sa_proj, residual ----
        for tok in range(3):
            b0 = 32 * tok
            aos = {}
            for c in range(NCH):
                ts = []
                for j in range(3):
                    pe = ps_big.tile([128, BF], F32, tag="big")
                    nc.tensor.matmul(pe[:], selb_c(c, b0),
                                     P[j][b0:b0 + 16, :], start=True, stop=True)
                    t = tt_pool.tile([128, BF], BF16, tag="tt")
                    nc.vector.tensor_tensor(t[:], pe[:], vs[(j, c)][:], op=OP.mult)
                    ts.append(t)
                ao = ao_pool.tile([128, BF], BF16, tag=f"ao_{c}")
                nc.vector.tensor_tensor(ao[:], ts[0][:], ts[1][:], op=OP.add)
                nc.vector.tensor_tensor(ao[:], ao[:], ts[2][:], op=OP.add)
                aos[c] = ao
            # sa_proj for this token + bias + residual (in-place into xs)
            for og in range(4):
                wt = wog_pool.tile([128, NCH * 256], BF16, tag="wog",
                                   name=f"wsa_{tok}_{og}")
                wsl = ins["wsa_p"][:, og * NCH * 256:(og + 1) * NCH * 256]
                nc.sync.dma_start(wt[:], wsl)
                wtv = wt[:].rearrange("p (k n) -> p k n", k=NCH)
                for jj in range(2):
                    o = og * 2 + jj
                    ps = ps_big.tile([128, BF], F32, tag="big")
                    for k in range(NCH):
                        nc.tensor.matmul(ps[:], wtv[:, k, jj * 128:(jj + 1) * 128],
                                         aos[k][:], start=(k == 0), stop=(k == NCH - 1))
                    nc.vector.scalar_tensor_tensor(
                        xs[(tok, o)][:], ps[:], cols["sab"][:, o:o + 1],
                        xs[(tok, o)][:], op0=OP.add, op1=OP.add)

        # ---- phase 5: LN1, 3 tokens packed ----
        ln_group([[xs[(tok, c)][:] for c in range(NCH)] for tok in range(3)],
                 cols["l1g"], cols["l1b"],
                 [[xs[(tok, c)][:] for c in range(NCH)] for tok in range(3)])

        # ---- phase 6: cross attention ----
        # cq = t_text @ Wq (scale folded host-side)
        cqs = {}
        for og in range(4):
            wt = wog_pool.tile([128, NCH * 256], BF16, tag="wog", name="w_wq_og")
            wsl = ins["wq_p"][:, og * NCH * 256:(og + 1) * NCH * 256]
            nc.sync.dma_start(wt[:], wsl)
            wtv = wt[:].rearrange("p (k n) -> p k n", k=NCH)
            for jj in range(2):
                o = og * 2 + jj
                ps = ps_big.tile([128, BF], F32, tag="big")
                for k in range(NCH):
                    nc.tensor.matmul(ps[:], wtv[:, k, jj * 128:(jj + 1) * 128],
                                     xs[(0, k)][:], start=(k == 0), stop=(k == NCH - 1))
                cq = cq_pool.tile([128, BF], BF16, tag=f"cq_{o}")
                nc.scalar.copy(cq[:], ps[:])
                cqs[o] = cq
        # ck for img(tok1), aud(tok2): Wkv og 0..3 (cols 0..1023)
        Sc0 = ps_S.tile([128, BF], F32, tag="S", name="Sc0")
        Sc1 = ps_S.tile([128, BF], F32, tag="S", name="Sc1")
        Scs = [Sc0, Sc1]
        n_seen = [0, 0]  # per jj-pair chunk counter for S accumulation
        for og in range(4):
            wt = wog_pool.tile([128, NCH * 256], BF16, tag="wog", name="w_wk_og")
            wsl = ins["wkv_p"][:, og * NCH * 256:(og + 1) * NCH * 256]
            nc.sync.dma_start(wt[:], wsl)
            wtv = wt[:].rearrange("p (k n) -> p k n", k=NCH)
            for tok in (1, 2):
                for jj in range(2):
                    c = og * 2 + jj
                    ps = ps_big.tile([128, BF], F32, tag="big")
                    for k in range(NCH):
                        nc.tensor.matmul(ps[:], wtv[:, k, jj * 128:(jj + 1) * 128],
                                         xs[(tok, k)][:], start=(k == 0), stop=(k == NCH - 1))
                    ck = ck_pool.tile([128, BF], BF16, tag="ck")
                    nc.scalar.copy(ck[:], ps[:])
                    pi = tok - 1  # 0 = img, 1 = aud
                    pr = pr_pool.tile([128, BF], BF16, tag="prod")
                    nc.vector.tensor_tensor(pr[:], cqs[c][:], ck[:], op=OP.mult)
                    nc.tensor.matmul(
                        Scs[pi][0:16, :], sel_c(c), pr[:],
                        start=(n_seen[pi] == 0), stop=(n_seen[pi] == NCH - 1))
                    n_seen[pi] += 1
        # cross softmax over 2 keys (tags shared with self-softmax tiles,
        # lifetimes are disjoint within a block)
        Ec0 = sm_pool.tile([16, BF], F32, tag="E0", name="Ec0")
        nc.scalar.activation(Ec0[:], Sc0[0:16, :], AF.Exp)
        Ec1 = sm_pool.tile([16, BF], F32, tag="E1", name="Ec1")
        nc.scalar.activation(Ec1[:], Sc1[0:16, :], AF.Exp)
        esc = sm_pool.tile([16, BF], F32, tag="esum", name="esc")
        nc.vector.tensor_tensor(esc[:], Ec0[:], Ec1[:], op=OP.add)
        recc = sm_pool.tile([16, BF], F32, tag="rec", name="recc")
        nc.vector.reciprocal_approx_fast(recc[:], esc[:])
        Pc0 = pp_pool.tile([16, BF], BF16, tag="P0", name="Pc0")
        nc.vector.tensor_tensor(Pc0[:], Ec0[:], recc[:], op=OP.mult)
        Pc1 = pp_pool.tile([16, BF], BF16, tag="P1", name="Pc1")
        nc.vector.tensor_tensor(Pc1[:], Ec1[:], recc[:], op=OP.mult)
        # cv for img/aud: Wkv og 4..7 (cols 1024..2047)
        cvs = {}
        for og in range(4, 8):
            wt = wog_pool.tile([128, NCH * 256], BF16, tag="wog", name="w_wv_og")
            wsl = ins["wkv_p"][:, og * NCH * 256:(og + 1) * NCH * 256]
            nc.sync.dma_start(wt[:], wsl)
            wtv = wt[:].rearrange("p (k n) -> p k n", k=NCH)
            for tok in (1, 2):
                for jj in range(2):
                    c = (og - 4) * 2 + jj
                    ps = ps_big.tile([128, BF], F32, tag="big")
                    for k in range(NCH):
                        nc.tensor.matmul(ps[:], wtv[:, k, jj * 128:(jj + 1) * 128],
                                         xs[(tok, k)][:], start=(k == 0), stop=(k == NCH - 1))
                    cv = cv_pool.tile([128, BF], BF16, tag=f"cv_{tok}_{c}")
                    nc.scalar.copy(cv[:], ps[:])
                    cvs[(tok, c)] = cv
        # weighted cv sum -> cross attnout
        caos = {}
        for c in range(NCH):
            pe_i = ps_big.tile([128, BF], F32, tag="big")
            nc.tensor.matmul(pe_i[:], selb_c(c), Pc0[:], start=True, stop=True)
            pe_a = ps_big.tile([128, BF], F32, tag="big")
            nc.tensor.matmul(pe_a[:], selb_c(c), Pc1[:], start=True, stop=True)
            t0 = tt_pool.tile([128, BF], BF16, tag="tt")
            nc.vector.tensor_tensor(t0[:], pe_i[:], cvs[(1, c)][:], op=OP.mult)
            t1 = tt_pool.tile([128, BF], BF16, tag="tt")
            nc.vector.tensor_tensor(t1[:], pe_a[:], cvs[(2, c)][:], op=OP.mult)
            cao = ao_pool.tile([128, BF], BF16, tag=f"ao_{c}", name=f"cao_{c}")
            nc.vector.tensor_tensor(cao[:], t0[:], t1[:], op=OP.add)
            caos[c] = cao
        # ca_proj + bias + residual (in-place into text xs)
        for og in range(4):
            wt = wog_pool.tile([128, NCH * 256], BF16, tag="wog", name="w_wca_og")
            wsl = ins["wca_p"][:, og * NCH * 256:(og + 1) * NCH * 256]
            nc.sync.dma_start(wt[:], wsl)
            wtv = wt[:].rearrange("p (k n) -> p k n", k=NCH)
            for jj in range(2):
                o = og * 2 + jj
                ps = ps_big.tile([128, BF], F32, tag="big")
                for k in range(NCH):
                    nc.tensor.matmul(ps[:], wtv[:, k, jj * 128:(jj + 1) * 128],
                                     caos[k][:], start=(k == 0), stop=(k == NCH - 1))
                nc.vector.scalar_tensor_tensor(
                    xs[(0, o)][:], ps[:], cols["cab"][:, o:o + 1],
                    xs[(0, o)][:], op0=OP.add, op1=OP.add)

        # ---- phase 7: LN2 on text token, f32 out, store ----
        outs_t = [out_pool.tile([128, BF], F32, tag=f"out_{c % 2}", name=f"out_{c}")
                  for c in range(NCH)]
        ln_group([[xs[(0, c)][:] for c in range(NCH)]],
                 cols["l2g"], cols["l2b"], [[o[:] for o in outs_t]])
        for c in range(NCH):
            nc.sync.dma_start(out_t[c * 128:(c + 1) * 128, bs:bs + BF], outs_t[c][:])

    for blk in range(nblk):
        tail(head(blk))

    stack.close()


# ------------------------------------------------------------------ host side

def _prep_shared(Wqkv, sa_proj_w, sa_proj_b, ln1_g, ln1_b, Wq, Wkv, ca_proj_w,
                 ca_proj_b, ln2_g, ln2_b):
    f = np.float32
    bf = ml_dtypes.bfloat16

    def kperm(W):  # [1024, N] -> [128, og, 8k, 256] flat (og-contiguous slabs)
        N = W.shape[1]
        kp = W.reshape(NCH, 128, N).transpose(1, 0, 2)          # [128, 8k, N]
        nog = N // 256
        og = kp.reshape(128, NCH, nog, 256).transpose(0, 2, 1, 3)  # [128, og, k, 256]
        return np.ascontiguousarray(og.reshape(128, N * NCH)).astype(bf)

    # Wqkv: per out-chunk c: [q_c | k_c | v_c] columns adjacent; SCALE folded
    # into the q block (so scores need no extra scaling).
    Wq3 = np.asarray(Wqkv, f).reshape(1024, 3, NCH, 128).copy()  # [k, qkv, c, 128]
    Wq3[:, 0] *= SCALE
    per_c = []
    for c in range(NCH):
        colsq = np.concatenate([Wq3[:, t, c, :] for t in range(3)], axis=1)  # [1024,384]
        per_c.append(colsq.reshape(NCH, 128, 384).transpose(1, 0, 2).reshape(128, NCH * 384))
    wqkv_p = np.ascontiguousarray(np.concatenate(per_c, axis=1)).astype(bf)

    sel = np.zeros((128, NCH, 16), f)
    for r in range(128):
        for c in range(NCH):
            sel[r, c, 2 * c + r // 64] = 1.0
    selb1 = np.zeros((16, NCH, 128), f)
    for h in range(16):
        for c in range(NCH):
            for m in range(128):
                if h == 2 * c + m // 64:
                    selb1[h, c, m] = 1.0
    selb = np.zeros((96, NCH, 128), f)
    for b0 in (0, 32, 64):
        selb[b0:b0 + 16] = selb1
    onesrow3 = np.zeros((65, 128), f)
    for b0 in (0, 32, 64):
        onesrow3[b0] = 1.0
    col = lambda v: np.ascontiguousarray(np.asarray(v, f).reshape(NCH, 128).T)
    return {
        "wqkv_p": wqkv_p,
        "wsa_p": kperm(np.asarray(sa_proj_w, f)),
        "wq_p": kperm(np.asarray(Wq, f) * SCALE),
        "wkv_p": kperm(np.asarray(Wkv, f)),
        "wca_p": kperm(np.asarray(ca_proj_w, f)),
        "sel": sel.reshape(128, NCH * 16).astype(bf),
        "selb": selb.reshape(96, NCH * 128).astype(bf),
        "ones_col": np.full((128, 1), 1.0 / D, f).astype(bf),
        "ones_row": onesrow3.astype(bf),
        "sab": col(sa_proj_b), "l1g": col(ln1_g), "l1b": col(ln1_b),
        "cab": col(ca_proj_b), "l2g": col(ln2_g), "l2b": col(ln2_b),
    }


_CACHE = {}


def _pin_act_tables(arch):
    """Force Exp and Ln to resolve to the one act-table set that holds both
    (natural_log_exp_and_others), so the kernel needs a single table load
    instead of thrashing between the exp- and ln-only sets. Mutates the
    cached membership sets in place; set ids/order are untouched."""
    from concourse import hw_specs
    tabs = hw_specs.get_activation_tables(arch)
    if "natural_log_exp_and_others" not in tabs:
        return
    for name, s in tabs.items():
        if name != "natural_log_exp_and_others":
            s.discard(AF.Exp)
            s.discard(AF.Ln)


def _get_program(nblk):
    if nblk in _CACHE:
        return _CACHE[nblk]
    nc = bacc.Bacc("TRN2", target_bir_lowering=False, debug=False,
                   enable_asserts=False, num_devices=NCORES)
    _pin_act_tables(nc.m.arch)
    ins = {}
    bl = nblk * BF
    for nm in ("xt_t", "xt_i", "xt_a"):
        ins[nm] = nc.dram_tensor(nm, [D, bl], BF16, kind="ExternalInput").ap()
    ins["wqkv_p"] = nc.dram_tensor("wqkv_p", [128, NCH * NCH * 384], BF16, kind="ExternalInput").ap()
    for nm, w in (("wsa_p", 1024), ("wq_p", 1024), ("wkv_p", 2048), ("wca_p", 1024)):
        ins[nm] = nc.dram_tensor(nm, [128, NCH * w], BF16, kind="ExternalInput").ap()
    ins["sel"] = nc.dram_tensor("sel", [128, NCH * 16], BF16, kind="ExternalInput").ap()
    ins["selb"] = nc.dram_tensor("selb", [96, NCH * 128], BF16, kind="ExternalInput").ap()
    ins["ones_col"] = nc.dram_tensor("ones_col", [128, 1], BF16, kind="ExternalInput").ap()
    ins["ones_row"] = nc.dram_tensor("ones_row", [65, 128], BF16, kind="ExternalInput").ap()
    for nm in ("sab", "l1g", "l1b", "cab", "l2g", "l2b"):
        ins[nm] = nc.dram_tensor(nm, [128, NCH], F32, kind="ExternalInput").ap()
    outs = {"out_t": nc.dram_tensor("out_t", [D, bl], F32, kind="ExternalOutput").ap()}

    with tile.TileContext(nc) as tc:
        build(tc, outs, ins, nblk)
    nc.compile()
    _CACHE[nblk] = nc
    return nc


def kernel(c_text, c_image, c_audio, Wqkv, sa_proj_w, sa_proj_b, ln1_g, ln1_b,
           Wq, Wkv, ca_proj_w, ca_proj_b, ln2_g, ln2_b, _trace=False):
    bf = ml_dtypes.bfloat16
    shared = _prep_shared(Wqkv, sa_proj_w, sa_proj_b, ln1_g, ln1_b, Wq, Wkv,
                          ca_proj_w, ca_proj_b, ln2_g, ln2_b)
    xT = {
        "xt_t": np.ascontiguousarray(np.asarray(c_text, np.float32).T).astype(bf),
        "xt_i": np.ascontiguousarray(np.asarray(c_image, np.float32).T).astype(bf),
        "xt_a": np.ascontiguousarray(np.asarray(c_audio, np.float32).T).astype(bf),
    }
    in_maps = []
    for s in range(NCORES):
        sl = slice(s * BLOC, (s + 1) * BLOC)
        m = dict(shared)
        for k in xT:
            m[k] = np.ascontiguousarray(xT[k][:, sl])
        in_maps.append(m)
    nc = _get_program(NBLK_HW)
    res = bass_utils.run_bass_kernel_spmd(nc, in_maps, core_ids=list(range(NCORES)),
                                          trace=_trace)
    out = np.concatenate([np.asarray(r["out_t"]).T for r in res.results], axis=0)
    if _trace:
        kernel.last_results = res
    return out.astype(np.float32)


# revision 44
# speedup vs baseline: 1.1390x; 1.1390x over previous
"""Trainium2 Bass kernel for nn_MCFL_49254684950998 (dense multimodal transformer block).

Strategy: pure data parallel over 8 NeuronCores (batch 16384 -> 2048/core).
Feature-major layout ("T layout": [feat_chunk=128, batch]); host pre-transposes
inputs and post-transposes outputs. All GEMMs run in bf16 (full PE rate,
fp32 PSUM accumulate). Attention (3-token self-attn + 2-token cross-attn)
uses DVE bf16 elementwise products + selection matmuls on the TensorEngine;
softmax and LayerNorm reciprocals via ACT ln/exp (one act table, no swaps);
LayerNorm stats via col-packed ones-matmuls, apply via rank-1 PE broadcasts
plus fused tensor_scalar. Buffers are pool-allocated with bufs>=2 on the
block-critical paths so the Tile scheduler overlaps adjacent batch blocks
(keeps PE warm / HAM un-throttled).
"""

import sys

sys.path.insert(0, "/opt/trn_rl_repo")

import numpy as np
import ml_dtypes

import concourse.bass as bass
import concourse.bacc as bacc
import concourse.tile as tile
import concourse.mybir as mybir
from concourse import bass_utils

F32 = mybir.dt.float32
BF16 = mybir.dt.bfloat16
AF = mybir.ActivationFunctionType
OP = mybir.AluOpType

B, D, H, HD = 16384, 1024, 16, 64
NCORES = 8
BLOC = B // NCORES          # 2048 batch rows per core
BF = 512                    # batch tile (free dim) per block
NBLK_HW = BLOC // BF        # 4 blocks per core
NCH = D // 128              # 8 feature chunks
SCALE = HD ** -0.5
EPS = 1e-5


def build(tc, outs, ins, nblk):
    from contextlib import ExitStack
    stack = ExitStack()
    nc = tc.nc
    out_t = outs["out_t"]
    xt = [ins["xt_t"], ins["xt_i"], ins["xt_a"]]

    # ---- const tiles (loaded once) ----
    cpool = stack.enter_context(tc.tile_pool(name="consts", bufs=1))
    sel_sb = cpool.tile([128, NCH * 16], BF16, tag="sel")       # [128, c, 16]
    nc.sync.dma_start(sel_sb[:], ins["sel"][:])
    selb_sb = cpool.tile([96, NCH * 128], BF16, tag="selb")     # bases 0/32/64
    nc.sync.dma_start(selb_sb[:], ins["selb"][:])
    ones_sb = cpool.tile([128, 1], BF16, tag="ones")            # 1/1024
    nc.sync.dma_start(ones_sb[:], ins["ones_col"][:])
    onesrow_sb = cpool.tile([65, 128], BF16, tag="onesrow")     # 1.0 @ rows 0/32/64
    nc.sync.dma_start(onesrow_sb[:], ins["ones_row"][:])
    cols = {}
    for nm in ("sab", "l1g", "l1b", "cab", "l2g", "l2b"):
        cols[nm] = cpool.tile([128, NCH], F32, tag=nm, name=f"col_{nm}")
        nc.sync.dma_start(cols[nm][:], ins[nm][:])

    def sel_c(c):
        return sel_sb[:, c * 16:(c + 1) * 16]

    def selb_c(c, base=0):
        return selb_sb[base:base + 16, c * 128:(c + 1) * 128]

    # ---- pools ----
    def pool(*a, **k):
        return stack.enter_context(tc.tile_pool(*a, **k))

    wq_pool = pool(name="wqkv", bufs=3)       # [128, 8*384] bf16 (6KB)
    wog_pool = pool(name="wog", bufs=3)       # [128, 8*256] bf16 (4KB)
    x_pool = pool(name="xs", bufs=2)          # 24 tags [128,BF] bf16
    qk_pool = pool(name="qk", bufs=8)         # [128, BF] bf16
    v_pool = pool(name="vs", bufs=1)          # 24 tags [128,BF] bf16
    pr_pool = pool(name="prod", bufs=4)       # [128, BF] bf16
    tt_pool = pool(name="tt", bufs=4)         # [128, BF] bf16 (attnout terms)
    ao_pool = pool(name="ao", bufs=2)         # 8 tags [128,BF] bf16
    cq_pool = pool(name="cq", bufs=1)         # 8 tags [128, BF] bf16
    ck_pool = pool(name="ck", bufs=4)         # [128, BF] bf16
    cv_pool = pool(name="cv", bufs=1)         # 16 tags [128,BF] bf16
    sq_pool = pool(name="sq", bufs=2)         # [128, BF] bf16 squares
    sm_pool = pool(name="sm", bufs=1)         # small f32/bf16 softmax+LN tiles
    pp_pool = pool(name="pp", bufs=2)         # softmax P tiles (cross-block)
    out_pool = pool(name="outp", bufs=1)      # 2 tags [128,BF] f32
    rbc_pool = pool(name="rbc", bufs=2)       # [128,BF] bf16 rstd broadcast
    ps_big = pool(name="psbig", bufs=5, space="PSUM")
    ps_S = pool(name="psS", bufs=3, space="PSUM")

    def ln_group(ylists, gcol, bcol, dstlists):
        """Layernorm over the feature (partition-chunk) dim for 1-3 tokens at
        once. ylists/dstlists: per-token lists of 8 [128,BF] APs. Token t's
        stats live at partition 32t of two PSUM banks (col-group packing);
        smalls processed on rows [0:R] in one shot (junk rows harmless)."""
        ntok = len(ylists)
        R = 32 * (ntok - 1) + 1
        stA = ps_S.tile([128, BF], F32, tag="S", name="stA")
        stB = ps_S.tile([128, BF], F32, tag="S", name="stB")
        for t in range(ntok):
            b0 = 32 * t
            for c in range(NCH):
                sq = sq_pool.tile([128, BF], BF16, tag="sq")
                nc.gpsimd.tensor_tensor(sq[:], ylists[t][c], ylists[t][c],
                                        op=OP.mult)
                st, sp = (c == 0), (c == NCH - 1)
                nc.tensor.matmul(stA[b0:b0 + 1, :], ones_sb[:], ylists[t][c],
                                 start=st, stop=sp, tile_position=(0, b0),
                                 skip_group_check=True)
                nc.tensor.matmul(stB[b0:b0 + 1, :], ones_sb[:], sq[:],
                                 start=st, stop=sp, tile_position=(0, b0),
                                 skip_group_check=True)
        mu_s = sm_pool.tile([65, BF], F32, tag="mu_s")
        nc.vector.tensor_copy(mu_s[0:R, :], stA[0:R, :])
        mu2 = sm_pool.tile([65, BF], F32, tag="lnvr", name="mu2")
        nc.vector.tensor_tensor(mu2[0:R, :], mu_s[0:R, :], mu_s[0:R, :], op=OP.mult)
        var = sm_pool.tile([65, BF], F32, tag="var")
        nc.vector.scalar_tensor_tensor(var[0:R, :], stB[0:R, :], EPS, mu2[0:R, :],
                                       op0=OP.add, op1=OP.subtract)
        lnv = sm_pool.tile([65, BF], F32, tag="lnvr")
        nc.scalar.activation(lnv[0:R, :], var[0:R, :], AF.Ln)
        rstd = sm_pool.tile([65, BF], BF16, tag="rstd")
        nc.scalar.activation(rstd[0:R, :], lnv[0:R, :], AF.Exp, scale=-0.5)
        mup = sm_pool.tile([65, BF], BF16, tag="mup")
        nc.vector.tensor_tensor(mup[0:R, :], mu_s[0:R, :], rstd[0:R, :], op=OP.mult)
        for t in range(ntok):
            b0 = 32 * t
            rb_ps = ps_S.tile([128, BF], F32, tag="S", name="rb_ps")
            nc.tensor.matmul(rb_ps[:], onesrow_sb[b0:b0 + 1, :],
                             rstd[b0:b0 + 1, :], start=True, stop=True)
            rb = rbc_pool.tile([128, BF], BF16, tag="rbc")
            nc.scalar.copy(rb[:], rb_ps[:])
            mu_ps = ps_S.tile([128, BF], F32, tag="S", name="mu_ps")
            nc.tensor.matmul(mu_ps[:], onesrow_sb[b0:b0 + 1, :],
                             mup[b0:b0 + 1, :], start=True, stop=True)
            for c in range(NCH):
                t1 = tt_pool.tile([128, BF], BF16, tag="tt")
                nc.vector.tensor_tensor(t1[:], ylists[t][c], rb[:], op=OP.mult)
                t2 = tt_pool.tile([128, BF], BF16, tag="tt")
                nc.vector.tensor_tensor(t2[:], t1[:], mu_ps[:], op=OP.subtract)
                nc.vector.tensor_scalar(dstlists[t][c], t2[:], gcol[:, c:c + 1],
                                        bcol[:, c:c + 1], op0=OP.mult, op1=OP.add)

    def head(blk):
        """Phases 1-3: x load, qkv GEMM + scores, softmax -> P tiles."""
        bs = blk * BF
        # first weight slab before the x queue so the PE isn't DMA-gated
        wt0 = wq_pool.tile([128, NCH * 384], BF16, tag="wqkv", name="wt0")
        wsl0 = ins["wqkv_p"][:, 0:NCH * 384]
        half = NCH * 192
        nc.sync.dma_start(wt0[:, :half], wsl0[:, :half])
        nc.scalar.dma_start(wt0[:, half:], wsl0[:, half:])
        xs = {}
        qs = (nc.sync, nc.scalar)
        for m in range(3):
            for c in range(NCH):
                t = x_pool.tile([128, BF], BF16, tag=f"x_{m}_{c}")
                qs[(m * NCH + c) % 2].dma_start(
                    t[:], xt[m][c * 128:(c + 1) * 128, bs:bs + BF])
                xs[(m, c)] = t

        S_banks = [ps_S.tile([128, BF], F32, tag="S", name=f"Sbank{i}")
                   for i in range(3)]
        vs = {}

        def emit_products(c, qts, kts):
            for i in range(3):
                for j in range(3):
                    pr = pr_pool.tile([128, BF], BF16, tag="prod")
                    nc.vector.tensor_tensor(pr[:], qts[i][:], kts[j][:], op=OP.mult)
                    nc.tensor.matmul(
                        S_banks[j][32 * i:32 * i + 16, :],
                        sel_c(c), pr[:],
                        start=(c == 0), stop=(c == NCH - 1),
                        tile_position=(0, 32 * i),
                        skip_group_check=True,
                    )

        for c in range(NCH):
            if c == 0:
                wt = wt0
            else:
                wt = wq_pool.tile([128, NCH * 384], BF16, tag="wqkv")
                wsl = ins["wqkv_p"][:, c * NCH * 384:(c + 1) * NCH * 384]
                nc.sync.dma_start(wt[:, :half], wsl[:, :half])
                nc.scalar.dma_start(wt[:, half:], wsl[:, half:])
            qts, kts = [], []
            for m in range(3):
                psq = ps_big.tile([128, BF], F32, tag="big")
                psk = ps_big.tile([128, BF], F32, tag="big")
                psv = ps_big.tile([128, BF], F32, tag="big")
                for k in range(NCH):
                    wk = wt[:, k * 384:(k + 1) * 384]
                    st, sp = (k == 0), (k == NCH - 1)
                    nc.tensor.matmul(psq[:], wk[:, 0:128], xs[(m, k)][:], start=st, stop=sp)
                    nc.tensor.matmul(psk[:], wk[:, 128:256], xs[(m, k)][:], start=st, stop=sp)
                    nc.tensor.matmul(psv[:], wk[:, 256:384], xs[(m, k)][:], start=st, stop=sp)
                qt = qk_pool.tile([128, BF], BF16, tag="qk")
                kt = qk_pool.tile([128, BF], BF16, tag="qk")
                vt = v_pool.tile([128, BF], BF16, tag=f"v_{m}_{c}")
                nc.scalar.copy(qt[:], psq[:])
                nc.scalar.copy(kt[:], psk[:])
                nc.scalar.copy(vt[:], psv[:])
                qts.append(qt)
                kts.append(kt)
                vs[(m, c)] = vt
            emit_products(c, qts, kts)

        def warm(src):
            """Tiny matmul chained on `src` ([80,BF] tile) into unused rows of
            S bank 0 — keeps the PE HAM window busy through the softmax
            serial chain (a >3.4us PE-idle gap would re-throttle the clock).
            Values are garbage (bitcast) — the output rows are never read."""
            mv = src[:].bitcast(BF16) if src.dtype == F32 else src[:]
            nc.tensor.matmul(S_banks[0][96:112, 0:64], sel_sb[0:80, 0:16],
                             mv[0:80, 0:64], start=True, stop=True,
                             tile_position=(0, 96), skip_group_check=True)

        # softmax over 3 keys; tokens packed at partition bases 0/32/64
        E = []
        for j in range(3):
            e = sm_pool.tile([80, BF], F32, tag=f"E{j}", name=f"E{j}")
            nc.scalar.activation(e[:], S_banks[j][0:80, :], AF.Exp)
            E.append(e)
        warm(E[0])
        esum = sm_pool.tile([80, BF], F32, tag="esum")
        nc.vector.tensor_tensor(esum[:], E[0][:], E[1][:], op=OP.add)
        nc.vector.tensor_tensor(esum[:], esum[:], E[2][:], op=OP.add)
        warm(esum)
        rec = sm_pool.tile([80, BF], F32, tag="rec")
        nc.vector.reciprocal_approx_fast(rec[:], esum[:])
        warm(rec)
        P = []
        for j in range(3):
            p = pp_pool.tile([80, BF], BF16, tag=f"P{j}", name=f"P{j}")
            nc.vector.tensor_tensor(p[:], E[j][:], rec[:], op=OP.mult)
            P.append(p)
        return dict(bs=bs, xs=xs, vs=vs, P=P)

    def tail(st):
        """Phases 4-7 for a block whose head already ran."""
        bs, xs, vs, P = st["bs"], st["xs"], st["vs"], st["P"]
        # ---- phase 4: per token: weighted sum, sa_proj, residual ----
        for tok in range(3):
            b0 = 32 * tok
            aos = {}
            for c in range(NCH):
                ts = []
                for j in range(3):
                    pe = ps_big.tile([128, BF], F32, tag="big")
                    nc.tensor.matmul(pe[:], selb_c(c, b0),
                                     P[j][b0:b0 + 16, :], start=True, stop=True)
                    t = tt_pool.tile([128, BF], BF16, tag="tt")
                    nc.vector.tensor_tensor(t[:], pe[:], vs[(j, c)][:], op=OP.mult)
                    ts.append(t)
                ao = ao_pool.tile([128, BF], BF16, tag=f"ao_{c}")
                nc.vector.tensor_tensor(ao[:], ts[0][:], ts[1][:], op=OP.add)
                nc.vector.tensor_tensor(ao[:], ao[:], ts[2][:], op=OP.add)
                aos[c] = ao
            # sa_proj for this token + bias + residual (in-place into xs)
            for og in range(4):
                wt = wog_pool.tile([128, NCH * 256], BF16, tag="wog",
                                   name=f"wsa_{tok}_{og}")
                wsl = ins["wsa_p"][:, og * NCH * 256:(og + 1) * NCH * 256]
                nc.sync.dma_start(wt[:], wsl)
                wtv = wt[:].rearrange("p (k n) -> p k n", k=NCH)
                for jj in range(2):
                    o = og * 2 + jj
                    ps = ps_big.tile([128, BF], F32, tag="big")
                    for k in range(NCH):
                        nc.tensor.matmul(ps[:], wtv[:, k, jj * 128:(jj + 1) * 128],
                                         aos[k][:], start=(k == 0), stop=(k == NCH - 1))
                    nc.vector.scalar_tensor_tensor(
                        xs[(tok, o)][:], ps[:], cols["sab"][:, o:o + 1],
                        xs[(tok, o)][:], op0=OP.add, op1=OP.add)

        # ---- phase 5: LN1, 3 tokens packed ----
        ln_group([[xs[(tok, c)][:] for c in range(NCH)] for tok in range(3)],
                 cols["l1g"], cols["l1b"],
                 [[xs[(tok, c)][:] for c in range(NCH)] for tok in range(3)])

        # ---- phase 6: cross attention ----
        # cq = t_text @ Wq (scale folded host-side)
        cqs = {}
        for og in range(4):
            wt = wog_pool.tile([128, NCH * 256], BF16, tag="wog", name="w_wq_og")
            wsl = ins["wq_p"][:, og * NCH * 256:(og + 1) * NCH * 256]
            nc.sync.dma_start(wt[:], wsl)
            wtv = wt[:].rearrange("p (k n) -> p k n", k=NCH)
            for jj in range(2):
                o = og * 2 + jj
                ps = ps_big.tile([128, BF], F32, tag="big")
                for k in range(NCH):
                    nc.tensor.matmul(ps[:], wtv[:, k, jj * 128:(jj + 1) * 128],
                                     xs[(0, k)][:], start=(k == 0), stop=(k == NCH - 1))
                cq = cq_pool.tile([128, BF], BF16, tag=f"cq_{o}")
                nc.scalar.copy(cq[:], ps[:])
                cqs[o] = cq
        # ck for img(tok1), aud(tok2): Wkv og 0..3 (cols 0..1023)
        Sc0 = ps_S.tile([128, BF], F32, tag="S", name="Sc0")
        Sc1 = ps_S.tile([128, BF], F32, tag="S", name="Sc1")
        Scs = [Sc0, Sc1]
        n_seen = [0, 0]  # per jj-pair chunk counter for S accumulation
        for og in range(4):
            wt = wog_pool.tile([128, NCH * 256], BF16, tag="wog", name="w_wk_og")
            wsl = ins["wkv_p"][:, og * NCH * 256:(og + 1) * NCH * 256]
            nc.sync.dma_start(wt[:], wsl)
            wtv = wt[:].rearrange("p (k n) -> p k n", k=NCH)
            for tok in (1, 2):
                for jj in range(2):
                    c = og * 2 + jj
                    ps = ps_big.tile([128, BF], F32, tag="big")
                    for k in range(NCH):
                        nc.tensor.matmul(ps[:], wtv[:, k, jj * 128:(jj + 1) * 128],
                                         xs[(tok, k)][:], start=(k == 0), stop=(k == NCH - 1))
                    ck = ck_pool.tile([128, BF], BF16, tag="ck")
                    nc.scalar.copy(ck[:], ps[:])
                    pi = tok - 1  # 0 = img, 1 = aud
                    pr = pr_pool.tile([128, BF], BF16, tag="prod")
                    nc.vector.tensor_tensor(pr[:], cqs[c][:], ck[:], op=OP.mult)
                    nc.tensor.matmul(
                        Scs[pi][0:16, :], sel_c(c), pr[:],
                        start=(n_seen[pi] == 0), stop=(n_seen[pi] == NCH - 1))
                    n_seen[pi] += 1
        # cross softmax over 2 keys (tags shared with self-softmax tiles,
        # lifetimes are disjoint within a block)
        Ec0 = sm_pool.tile([16, BF], F32, tag="E0", name="Ec0")
        nc.scalar.activation(Ec0[:], Sc0[0:16, :], AF.Exp)
        Ec1 = sm_pool.tile([16, BF], F32, tag="E1", name="Ec1")
        nc.scalar.activation(Ec1[:], Sc1[0:16, :], AF.Exp)
        esc = sm_pool.tile([16, BF], F32, tag="esum", name="esc")
        nc.vector.tensor_tensor(esc[:], Ec0[:], Ec1[:], op=OP.add)
        recc = sm_pool.tile([16, BF], F32, tag="rec", name="recc")
        nc.vector.reciprocal_approx_fast(recc[:], esc[:])
        Pc0 = pp_pool.tile([16, BF], BF16, tag="P0", name="Pc0")
        nc.vector.tensor_tensor(Pc0[:], Ec0[:], recc[:], op=OP.mult)
        Pc1 = pp_pool.tile([16, BF], BF16, tag="P1", name="Pc1")
        nc.vector.tensor_tensor(Pc1[:], Ec1[:], recc[:], op=OP.mult)
        # cv for img/aud: Wkv og 4..7 (cols 1024..2047)
        cvs = {}
        for og in range(4, 8):
            wt = wog_pool.tile([128, NCH * 256], BF16, tag="wog", name="w_wv_og")
            wsl = ins["wkv_p"][:, og * NCH * 256:(og + 1) * NCH * 256]
            nc.sync.dma_start(wt[:], wsl)
            wtv = wt[:].rearrange("p (k n) -> p k n", k=NCH)
            for tok in (1, 2):
                for jj in range(2):
                    c = (og - 4) * 2 + jj
                    ps = ps_big.tile([128, BF], F32, tag="big")
                    for k in range(NCH):
                        nc.tensor.matmul(ps[:], wtv[:, k, jj * 128:(jj + 1) * 128],
                                         xs[(tok, k)][:], start=(k == 0), stop=(k == NCH - 1))
                    cv = cv_pool.tile([128, BF], BF16, tag=f"cv_{tok}_{c}")
                    nc.scalar.copy(cv[:], ps[:])
                    cvs[(tok, c)] = cv
        # weighted cv sum -> cross attnout
        caos = {}
        for c in range(NCH):
            pe_i = ps_big.tile([128, BF], F32, tag="big")
            nc.tensor.matmul(pe_i[:], selb_c(c), Pc0[:], start=True, stop=True)
            pe_a = ps_big.tile([128, BF], F32, tag="big")
            nc.tensor.matmul(pe_a[:], selb_c(c), Pc1[:], start=True, stop=True)
            t0 = tt_pool.tile([128, BF], BF16, tag="tt")
            nc.vector.tensor_tensor(t0[:], pe_i[:], cvs[(1, c)][:], op=OP.mult)
            t1 = tt_pool.tile([128, BF], BF16, tag="tt")
            nc.vector.tensor_tensor(t1[:], pe_a[:], cvs[(2, c)][:], op=OP.mult)
            cao = ao_pool.tile([128, BF], BF16, tag=f"ao_{c}", name=f"cao_{c}")
            nc.vector.tensor_tensor(cao[:], t0[:], t1[:], op=OP.add)
            caos[c] = cao
        # ca_proj + bias + residual (in-place into text xs)
        for og in range(4):
            wt = wog_pool.tile([128, NCH * 256], BF16, tag="wog", name="w_wca_og")
            wsl = ins["wca_p"][:, og * NCH * 256:(og + 1) * NCH * 256]
            nc.sync.dma_start(wt[:], wsl)
            wtv = wt[:].rearrange("p (k n) -> p k n", k=NCH)
            for jj in range(2):
                o = og * 2 + jj
                ps = ps_big.tile([128, BF], F32, tag="big")
                for k in range(NCH):
                    nc.tensor.matmul(ps[:], wtv[:, k, jj * 128:(jj + 1) * 128],
                                     caos[k][:], start=(k == 0), stop=(k == NCH - 1))
                nc.vector.scalar_tensor_tensor(
                    xs[(0, o)][:], ps[:], cols["cab"][:, o:o + 1],
                    xs[(0, o)][:], op0=OP.add, op1=OP.add)

        # ---- phase 7: LN2 on text token, f32 out, store ----
        outs_t = [out_pool.tile([128, BF], F32, tag=f"out_{c % 2}", name=f"out_{c}")
                  for c in range(NCH)]
        ln_group([[xs[(0, c)][:] for c in range(NCH)]],
                 cols["l2g"], cols["l2b"], [[o[:] for o in outs_t]])
        for c in range(NCH):
            nc.sync.dma_start(out_t[c * 128:(c + 1) * 128, bs:bs + BF], outs_t[c][:])

    for blk in range(nblk):
        tail(head(blk))

    stack.close()


# ------------------------------------------------------------------ host side

def _prep_shared(Wqkv, sa_proj_w, sa_proj_b, ln1_g, ln1_b, Wq, Wkv, ca_proj_w,
                 ca_proj_b, ln2_g, ln2_b):
    f = np.float32
    bf = ml_dtypes.bfloat16

    def kperm(W):  # [1024, N] -> [128, og, 8k, 256] flat (og-contiguous slabs)
        N = W.shape[1]
        kp = W.reshape(NCH, 128, N).transpose(1, 0, 2)          # [128, 8k, N]
        nog = N // 256
        og = kp.reshape(128, NCH, nog, 256).transpose(0, 2, 1, 3)  # [128, og, k, 256]
        return np.ascontiguousarray(og.reshape(128, N * NCH)).astype(bf)

    # Wqkv: per out-chunk c: [q_c | k_c | v_c] columns adjacent; SCALE folded
    # into the q block (so scores need no extra scaling).
    Wq3 = np.asarray(Wqkv, f).reshape(1024, 3, NCH, 128).copy()  # [k, qkv, c, 128]
    Wq3[:, 0] *= SCALE
    per_c = []
    for c in range(NCH):
        colsq = np.concatenate([Wq3[:, t, c, :] for t in range(3)], axis=1)  # [1024,384]
        per_c.append(colsq.reshape(NCH, 128, 384).transpose(1, 0, 2).reshape(128, NCH * 384))
    wqkv_p = np.ascontiguousarray(np.concatenate(per_c, axis=1)).astype(bf)

    sel = np.zeros((128, NCH, 16), f)
    for r in range(128):
        for c in range(NCH):
            sel[r, c, 2 * c + r // 64] = 1.0
    selb1 = np.zeros((16, NCH, 128), f)
    for h in range(16):
        for c in range(NCH):
            for m in range(128):
                if h == 2 * c + m // 64:
                    selb1[h, c, m] = 1.0
    selb = np.zeros((96, NCH, 128), f)
    for b0 in (0, 32, 64):
        selb[b0:b0 + 16] = selb1
    onesrow3 = np.zeros((65, 128), f)
    for b0 in (0, 32, 64):
        onesrow3[b0] = 1.0
    col = lambda v: np.ascontiguousarray(np.asarray(v, f).reshape(NCH, 128).T)
    return {
        "wqkv_p": wqkv_p,
        "wsa_p": kperm(np.asarray(sa_proj_w, f)),
        "wq_p": kperm(np.asarray(Wq, f) * SCALE),
        "wkv_p": kperm(np.asarray(Wkv, f)),
        "wca_p": kperm(np.asarray(ca_proj_w, f)),
        "sel": sel.reshape(128, NCH * 16).astype(bf),
        "selb": selb.reshape(96, NCH * 128).astype(bf),
        "ones_col": np.full((128, 1), 1.0 / D, f).astype(bf),
        "ones_row": onesrow3.astype(bf),
        "sab": col(sa_proj_b), "l1g": col(ln1_g), "l1b": col(ln1_b),
        "cab": col(ca_proj_b), "l2g": col(ln2_g), "l2b": col(ln2_b),
    }


_CACHE = {}


def _pin_act_tables(arch):
    """Force Exp and Ln to resolve to the one act-table set that holds both
    (natural_log_exp_and_others), so the kernel needs a single table load
    instead of thrashing between the exp- and ln-only sets. Mutates the
    cached membership sets in place; set ids/order are untouched."""
    from concourse import hw_specs
    tabs = hw_specs.get_activation_tables(arch)
    if "natural_log_exp_and_others" not in tabs:
        return
    for name, s in tabs.items():
        if name != "natural_log_exp_and_others":
            s.discard(AF.Exp)
            s.discard(AF.Ln)


def _get_program(nblk):
    if nblk in _CACHE:
        return _CACHE[nblk]
    nc = bacc.Bacc("TRN2", target_bir_lowering=False, debug=False,
                   enable_asserts=False, num_devices=NCORES)
    _pin_act_tables(nc.m.arch)
    ins = {}
    bl = nblk * BF
    for nm in ("xt_t", "xt_i", "xt_a"):
        ins[nm] = nc.dram_tensor(nm, [D, bl], BF16, kind="ExternalInput").ap()
    ins["wqkv_p"] = nc.dram_tensor("wqkv_p", [128, NCH * NCH * 384], BF16, kind="ExternalInput").ap()
    for nm, w in (("wsa_p", 1024), ("wq_p", 1024), ("wkv_p", 2048), ("wca_p", 1024)):
        ins[nm] = nc.dram_tensor(nm, [128, NCH * w], BF16, kind="ExternalInput").ap()
    ins["sel"] = nc.dram_tensor("sel", [128, NCH * 16], BF16, kind="ExternalInput").ap()
    ins["selb"] = nc.dram_tensor("selb", [96, NCH * 128], BF16, kind="ExternalInput").ap()
    ins["ones_col"] = nc.dram_tensor("ones_col", [128, 1], BF16, kind="ExternalInput").ap()
    ins["ones_row"] = nc.dram_tensor("ones_row", [65, 128], BF16, kind="ExternalInput").ap()
    for nm in ("sab", "l1g", "l1b", "cab", "l2g", "l2b"):
        ins[nm] = nc.dram_tensor(nm, [128, NCH], F32, kind="ExternalInput").ap()
    outs = {"out_t": nc.dram_tensor("out_t", [D, bl], F32, kind="ExternalOutput").ap()}

    with tile.TileContext(nc) as tc:
        build(tc, outs, ins, nblk)
    nc.compile()
    _CACHE[nblk] = nc
    return nc


def kernel(c_text, c_image, c_audio, Wqkv, sa_proj_w, sa_proj_b, ln1_g, ln1_b,
           Wq, Wkv, ca_proj_w, ca_proj_b, ln2_g, ln2_b, _trace=False):
    bf = ml_dtypes.bfloat16
    shared = _prep_shared(Wqkv, sa_proj_w, sa_proj_b, ln1_g, ln1_b, Wq, Wkv,
                          ca_proj_w, ca_proj_b, ln2_g, ln2_b)
    xT = {
        "xt_t": np.ascontiguousarray(np.asarray(c_text, np.float32).T).astype(bf),
        "xt_i": np.ascontiguousarray(np.asarray(c_image, np.float32).T).astype(bf),
        "xt_a": np.ascontiguousarray(np.asarray(c_audio, np.float32).T).astype(bf),
    }
    in_maps = []
    for s in range(NCORES):
        sl = slice(s * BLOC, (s + 1) * BLOC)
        m = dict(shared)
        for k in xT:
            m[k] = np.ascontiguousarray(xT[k][:, sl])
        in_maps.append(m)
    nc = _get_program(NBLK_HW)
    res = bass_utils.run_bass_kernel_spmd(nc, in_maps, core_ids=list(range(NCORES)),
                                          trace=_trace)
    out = np.concatenate([np.asarray(r["out_t"]).T for r in res.results], axis=0)
    if _trace:
        kernel.last_results = res
    return out.astype(np.float32)


# revision 45
# speedup vs baseline: 1.1902x; 1.0450x over previous
"""Trainium2 Bass kernel for nn_MCFL_49254684950998 (dense multimodal transformer block).

Strategy: pure data parallel over 8 NeuronCores (batch 16384 -> 2048/core).
Feature-major layout ("T layout": [feat_chunk=128, batch]); host pre-transposes
inputs and post-transposes outputs. All GEMMs run in bf16 (full PE rate,
fp32 PSUM accumulate). Attention (3-token self-attn + 2-token cross-attn)
uses DVE bf16 elementwise products + selection matmuls on the TensorEngine;
softmax and LayerNorm reciprocals via ACT ln/exp (one act table, no swaps);
LayerNorm stats via col-packed ones-matmuls, apply via rank-1 PE broadcasts
plus fused tensor_scalar. Buffers are pool-allocated with bufs>=2 on the
block-critical paths so the Tile scheduler overlaps adjacent batch blocks
(keeps PE warm / HAM un-throttled).
"""

import sys

sys.path.insert(0, "/opt/trn_rl_repo")

import numpy as np
import ml_dtypes

import concourse.bass as bass
import concourse.bacc as bacc
import concourse.tile as tile
import concourse.mybir as mybir
from concourse import bass_utils

F32 = mybir.dt.float32
BF16 = mybir.dt.bfloat16
AF = mybir.ActivationFunctionType
OP = mybir.AluOpType

B, D, H, HD = 16384, 1024, 16, 64
NCORES = 8
BLOC = B // NCORES          # 2048 batch rows per core
BF = 512                    # batch tile (free dim) per block
NBLK_HW = BLOC // BF        # 4 blocks per core
NCH = D // 128              # 8 feature chunks
SCALE = HD ** -0.5
EPS = 1e-5


def build(tc, outs, ins, nblk):
    from contextlib import ExitStack
    stack = ExitStack()
    nc = tc.nc
    out_t = outs["out_t"]
    xt = [ins["xt_t"], ins["xt_i"], ins["xt_a"]]

    # ---- const tiles (loaded once) ----
    cpool = stack.enter_context(tc.tile_pool(name="consts", bufs=1))
    sel_sb = cpool.tile([128, NCH * 16], BF16, tag="sel")       # [128, c, 16]
    nc.sync.dma_start(sel_sb[:], ins["sel"][:])
    selb_sb = cpool.tile([96, NCH * 128], BF16, tag="selb")     # bases 0/32/64
    nc.sync.dma_start(selb_sb[:], ins["selb"][:])
    ones_sb = cpool.tile([128, 1], BF16, tag="ones")            # 1/1024
    nc.sync.dma_start(ones_sb[:], ins["ones_col"][:])
    onesrow_sb = cpool.tile([65, 128], BF16, tag="onesrow")     # 1.0 @ rows 0/32/64
    nc.sync.dma_start(onesrow_sb[:], ins["ones_row"][:])
    cols = {}
    for nm in ("sab", "l1g", "l1b", "cab", "l2g", "l2b"):
        cols[nm] = cpool.tile([128, NCH], F32, tag=nm, name=f"col_{nm}")
        nc.sync.dma_start(cols[nm][:], ins[nm][:])

    def sel_c(c):
        return sel_sb[:, c * 16:(c + 1) * 16]

    def selb_c(c, base=0):
        return selb_sb[base:base + 16, c * 128:(c + 1) * 128]

    # ---- pools ----
    def pool(*a, **k):
        return stack.enter_context(tc.tile_pool(*a, **k))

    wq_pool = pool(name="wqkv", bufs=3)       # [128, 8*384] bf16 (6KB)
    wog_pool = pool(name="wog", bufs=3)       # [128, 8*256] bf16 (4KB)
    x_pool = pool(name="xs", bufs=2)          # 24 tags [128,BF] bf16
    qk_pool = pool(name="qk", bufs=8)         # [128, BF] bf16
    v_pool = pool(name="vs", bufs=1)          # 24 tags [128,BF] bf16
    pr_pool = pool(name="prod", bufs=4)       # [128, BF] bf16
    tt_pool = pool(name="tt", bufs=4)         # [128, BF] bf16 (attnout terms)
    ao_pool = pool(name="ao", bufs=2)         # 8 tags [128,BF] bf16
    cq_pool = pool(name="cq", bufs=1)         # 8 tags [128, BF] bf16
    ck_pool = pool(name="ck", bufs=4)         # [128, BF] bf16
    cv_pool = pool(name="cv", bufs=1)         # 16 tags [128,BF] bf16
    sq_pool = pool(name="sq", bufs=2)         # [128, BF] bf16 squares
    sm_pool = pool(name="sm", bufs=1)         # small f32/bf16 softmax+LN tiles
    pp_pool = pool(name="pp", bufs=2)         # softmax P tiles (cross-block)
    out_pool = pool(name="outp", bufs=1)      # 2 tags [128,BF] f32
    rbc_pool = pool(name="rbc", bufs=2)       # [128,BF] bf16 rstd broadcast
    ps_big = pool(name="psbig", bufs=5, space="PSUM")
    ps_S = pool(name="psS", bufs=3, space="PSUM")

    def ln_group(ylists, gcol, bcol, dstlists):
        """Layernorm over the feature (partition-chunk) dim for 1-3 tokens at
        once. ylists/dstlists: per-token lists of 8 [128,BF] APs. Token t's
        stats live at partition 32t of two PSUM banks (col-group packing);
        smalls processed on rows [0:R] in one shot (junk rows harmless)."""
        ntok = len(ylists)
        R = 32 * (ntok - 1) + 1
        stA = ps_S.tile([128, BF], F32, tag="S", name="stA")
        stB = ps_S.tile([128, BF], F32, tag="S", name="stB")
        for t in range(ntok):
            b0 = 32 * t
            for c in range(NCH):
                sq = sq_pool.tile([128, BF], BF16, tag="sq")
                nc.gpsimd.tensor_tensor(sq[:], ylists[t][c], ylists[t][c],
                                        op=OP.mult)
                st, sp = (c == 0), (c == NCH - 1)
                nc.tensor.matmul(stA[b0:b0 + 1, :], ones_sb[:], ylists[t][c],
                                 start=st, stop=sp, tile_position=(0, b0),
                                 skip_group_check=True)
                nc.tensor.matmul(stB[b0:b0 + 1, :], ones_sb[:], sq[:],
                                 start=st, stop=sp, tile_position=(0, b0),
                                 skip_group_check=True)
        mu_s = sm_pool.tile([65, BF], F32, tag="mu_s")
        nc.vector.tensor_copy(mu_s[0:R, :], stA[0:R, :])
        mu2 = sm_pool.tile([65, BF], F32, tag="lnvr", name="mu2")
        nc.vector.tensor_tensor(mu2[0:R, :], mu_s[0:R, :], mu_s[0:R, :], op=OP.mult)
        var = sm_pool.tile([65, BF], F32, tag="var")
        nc.vector.scalar_tensor_tensor(var[0:R, :], stB[0:R, :], EPS, mu2[0:R, :],
                                       op0=OP.add, op1=OP.subtract)
        lnv = sm_pool.tile([65, BF], F32, tag="lnvr")
        nc.scalar.activation(lnv[0:R, :], var[0:R, :], AF.Ln)
        rstd = sm_pool.tile([65, BF], BF16, tag="rstd")
        nc.scalar.activation(rstd[0:R, :], lnv[0:R, :], AF.Exp, scale=-0.5)
        mup = sm_pool.tile([65, BF], BF16, tag="mup")
        nc.vector.tensor_tensor(mup[0:R, :], mu_s[0:R, :], rstd[0:R, :], op=OP.mult)
        for t in range(ntok):
            b0 = 32 * t
            rb_ps = ps_S.tile([128, BF], F32, tag="S", name="rb_ps")
            nc.tensor.matmul(rb_ps[:], onesrow_sb[b0:b0 + 1, :],
                             rstd[b0:b0 + 1, :], start=True, stop=True)
            rb = rbc_pool.tile([128, BF], BF16, tag="rbc")
            nc.scalar.copy(rb[:], rb_ps[:])
            mu_ps = ps_S.tile([128, BF], F32, tag="S", name="mu_ps")
            nc.tensor.matmul(mu_ps[:], onesrow_sb[b0:b0 + 1, :],
                             mup[b0:b0 + 1, :], start=True, stop=True)
            for c in range(NCH):
                t1 = tt_pool.tile([128, BF], BF16, tag="tt")
                nc.vector.tensor_tensor(t1[:], ylists[t][c], rb[:], op=OP.mult)
                t2 = tt_pool.tile([128, BF], BF16, tag="tt")
                nc.vector.tensor_tensor(t2[:], t1[:], mu_ps[:], op=OP.subtract)
                nc.vector.tensor_scalar(dstlists[t][c], t2[:], gcol[:, c:c + 1],
                                        bcol[:, c:c + 1], op0=OP.mult, op1=OP.add)

    def head(blk):
        """Phases 1-3: x load, qkv GEMM + scores, softmax -> P tiles."""
        bs = blk * BF
        # first weight slab before the x queue so the PE isn't DMA-gated
        wt0 = wq_pool.tile([128, NCH * 384], BF16, tag="wqkv", name="wt0")
        wsl0 = ins["wqkv_p"][:, 0:NCH * 384]
        half = NCH * 192
        nc.sync.dma_start(wt0[:, :half], wsl0[:, :half])
        nc.scalar.dma_start(wt0[:, half:], wsl0[:, half:])
        xs = {}
        qs = (nc.sync, nc.scalar)
        for m in range(3):
            for c in range(NCH):
                t = x_pool.tile([128, BF], BF16, tag=f"x_{m}_{c}")
                qs[(m * NCH + c) % 2].dma_start(
                    t[:], xt[m][c * 128:(c + 1) * 128, bs:bs + BF])
                xs[(m, c)] = t

        S_banks = [ps_S.tile([128, BF], F32, tag="S", name=f"Sbank{i}")
                   for i in range(3)]
        vs = {}

        def emit_products(c, qts, kts):
            for i in range(3):
                for j in range(3):
                    pr = pr_pool.tile([128, BF], BF16, tag="prod")
                    nc.vector.tensor_tensor(pr[:], qts[i][:], kts[j][:], op=OP.mult)
                    nc.tensor.matmul(
                        S_banks[j][32 * i:32 * i + 16, :],
                        sel_c(c), pr[:],
                        start=(c == 0), stop=(c == NCH - 1),
                        tile_position=(0, 32 * i),
                        skip_group_check=True,
                    )

        for c in range(NCH):
            if c == 0:
                wt = wt0
            else:
                wt = wq_pool.tile([128, NCH * 384], BF16, tag="wqkv")
                wsl = ins["wqkv_p"][:, c * NCH * 384:(c + 1) * NCH * 384]
                nc.sync.dma_start(wt[:, :half], wsl[:, :half])
                nc.scalar.dma_start(wt[:, half:], wsl[:, half:])
            qts, kts = [], []
            for m in range(3):
                psq = ps_big.tile([128, BF], F32, tag="big")
                psk = ps_big.tile([128, BF], F32, tag="big")
                psv = ps_big.tile([128, BF], F32, tag="big")
                for k in range(NCH):
                    wk = wt[:, k * 384:(k + 1) * 384]
                    st, sp = (k == 0), (k == NCH - 1)
                    nc.tensor.matmul(psq[:], wk[:, 0:128], xs[(m, k)][:], start=st, stop=sp)
                    nc.tensor.matmul(psk[:], wk[:, 128:256], xs[(m, k)][:], start=st, stop=sp)
                    nc.tensor.matmul(psv[:], wk[:, 256:384], xs[(m, k)][:], start=st, stop=sp)
                qt = qk_pool.tile([128, BF], BF16, tag="qk")
                kt = qk_pool.tile([128, BF], BF16, tag="qk")
                vt = v_pool.tile([128, BF], BF16, tag=f"v_{m}_{c}")
                nc.scalar.copy(qt[:], psq[:])
                nc.scalar.copy(kt[:], psk[:])
                nc.scalar.copy(vt[:], psv[:])
                qts.append(qt)
                kts.append(kt)
                vs[(m, c)] = vt
            emit_products(c, qts, kts)

        def warm(src):
            """Tiny matmul chained on `src` ([80,BF] tile) into unused rows of
            S bank 0 — keeps the PE HAM window busy through the softmax
            serial chain (a >3.4us PE-idle gap would re-throttle the clock).
            Values are garbage (bitcast) — the output rows are never read."""
            mv = src[:].bitcast(BF16) if src.dtype == F32 else src[:]
            nc.tensor.matmul(S_banks[0][96:112, 0:64], sel_sb[0:80, 0:16],
                             mv[0:80, 0:64], start=True, stop=True,
                             tile_position=(0, 96), skip_group_check=True)

        # softmax over 3 keys; tokens packed at partition bases 0/32/64
        E = []
        for j in range(3):
            e = sm_pool.tile([80, BF], F32, tag=f"E{j}", name=f"E{j}")
            nc.scalar.activation(e[:], S_banks[j][0:80, :], AF.Exp)
            E.append(e)
        warm(E[0])
        esum = sm_pool.tile([80, BF], F32, tag="esum")
        nc.vector.tensor_tensor(esum[:], E[0][:], E[1][:], op=OP.add)
        nc.vector.tensor_tensor(esum[:], esum[:], E[2][:], op=OP.add)
        warm(esum)
        rec = sm_pool.tile([80, BF], F32, tag="rec")
        nc.vector.reciprocal_approx_fast(rec[:], esum[:])
        warm(rec)
        P = []
        for j in range(3):
            p = pp_pool.tile([80, BF], BF16, tag=f"P{j}", name=f"P{j}")
            nc.vector.tensor_tensor(p[:], E[j][:], rec[:], op=OP.mult)
            P.append(p)
        return dict(bs=bs, xs=xs, vs=vs, P=P)

    def tail(st):
        """Phases 4-7 for a block whose head already ran."""
        bs, xs, vs, P = st["bs"], st["xs"], st["vs"], st["P"]
        # ---- phase 4: per token: weighted sum, sa_proj, residual ----
        for tok in range(3):
            b0 = 32 * tok
            aos = {}
            for c in range(NCH):
                ts = []
                for j in range(3):
                    pe = ps_big.tile([128, BF], F32, tag="big")
                    nc.tensor.matmul(pe[:], selb_c(c, b0),
                                     P[j][b0:b0 + 16, :], start=True, stop=True)
                    t = tt_pool.tile([128, BF], BF16, tag="tt")
                    nc.vector.tensor_tensor(t[:], pe[:], vs[(j, c)][:], op=OP.mult)
                    ts.append(t)
                ao = ao_pool.tile([128, BF], BF16, tag=f"ao_{c}")
                nc.vector.tensor_tensor(ao[:], ts[0][:], ts[1][:], op=OP.add)
                nc.vector.tensor_tensor(ao[:], ao[:], ts[2][:], op=OP.add)
                aos[c] = ao
            # sa_proj for this token + bias + residual (in-place into xs)
            for og in range(4):
                wt = wog_pool.tile([128, NCH * 256], BF16, tag="wog",
                                   name=f"wsa_{tok}_{og}")
                wsl = ins["wsa_p"][:, og * NCH * 256:(og + 1) * NCH * 256]
                nc.sync.dma_start(wt[:], wsl)
                wtv = wt[:].rearrange("p (k n) -> p k n", k=NCH)
                for jj in range(2):
                    o = og * 2 + jj
                    ps = ps_big.tile([128, BF], F32, tag="big")
                    for k in range(NCH):
                        nc.tensor.matmul(ps[:], wtv[:, k, jj * 128:(jj + 1) * 128],
                                         aos[k][:], start=(k == 0), stop=(k == NCH - 1))
                    nc.vector.scalar_tensor_tensor(
                        xs[(tok, o)][:], ps[:], cols["sab"][:, o:o + 1],
                        xs[(tok, o)][:], op0=OP.add, op1=OP.add)

        # ---- phase 5: LN1, 3 tokens packed ----
        ln_group([[xs[(tok, c)][:] for c in range(NCH)] for tok in range(3)],
                 cols["l1g"], cols["l1b"],
                 [[xs[(tok, c)][:] for c in range(NCH)] for tok in range(3)])

        # ---- phase 6: cross attention ----
        # cq = t_text @ Wq (scale folded host-side)
        cqs = {}
        for og in range(4):
            wt = wog_pool.tile([128, NCH * 256], BF16, tag="wog", name="w_wq_og")
            wsl = ins["wq_p"][:, og * NCH * 256:(og + 1) * NCH * 256]
            nc.sync.dma_start(wt[:], wsl)
            wtv = wt[:].rearrange("p (k n) -> p k n", k=NCH)
            for jj in range(2):
                o = og * 2 + jj
                ps = ps_big.tile([128, BF], F32, tag="big")
                for k in range(NCH):
                    nc.tensor.matmul(ps[:], wtv[:, k, jj * 128:(jj + 1) * 128],
                                     xs[(0, k)][:], start=(k == 0), stop=(k == NCH - 1))
                cq = cq_pool.tile([128, BF], BF16, tag=f"cq_{o}")
                nc.scalar.copy(cq[:], ps[:])
                cqs[o] = cq
        # ck for img(tok1), aud(tok2): Wkv og 0..3 (cols 0..1023)
        Sc0 = ps_S.tile([128, BF], F32, tag="S", name="Sc0")
        Sc1 = ps_S.tile([128, BF], F32, tag="S", name="Sc1")
        Scs = [Sc0, Sc1]
        n_seen = [0, 0]  # per jj-pair chunk counter for S accumulation
        for og in range(4):
            wt = wog_pool.tile([128, NCH * 256], BF16, tag="wog", name="w_wk_og")
            wsl = ins["wkv_p"][:, og * NCH * 256:(og + 1) * NCH * 256]
            nc.sync.dma_start(wt[:], wsl)
            wtv = wt[:].rearrange("p (k n) -> p k n", k=NCH)
            for tok in (1, 2):
                for jj in range(2):
                    c = og * 2 + jj
                    ps = ps_big.tile([128, BF], F32, tag="big")
                    for k in range(NCH):
                        nc.tensor.matmul(ps[:], wtv[:, k, jj * 128:(jj + 1) * 128],
                                         xs[(tok, k)][:], start=(k == 0), stop=(k == NCH - 1))
                    ck = ck_pool.tile([128, BF], BF16, tag="ck")
                    nc.scalar.copy(ck[:], ps[:])
                    pi = tok - 1  # 0 = img, 1 = aud
                    pr = pr_pool.tile([128, BF], BF16, tag="prod")
                    nc.vector.tensor_tensor(pr[:], cqs[c][:], ck[:], op=OP.mult)
                    nc.tensor.matmul(
                        Scs[pi][0:16, :], sel_c(c), pr[:],
                        start=(n_seen[pi] == 0), stop=(n_seen[pi] == NCH - 1))
                    n_seen[pi] += 1
        # cross softmax over 2 keys (tags shared with self-softmax tiles,
        # lifetimes are disjoint within a block)
        Ec0 = sm_pool.tile([16, BF], F32, tag="E0", name="Ec0")
        nc.scalar.activation(Ec0[:], Sc0[0:16, :], AF.Exp)
        Ec1 = sm_pool.tile([16, BF], F32, tag="E1", name="Ec1")
        nc.scalar.activation(Ec1[:], Sc1[0:16, :], AF.Exp)
        esc = sm_pool.tile([16, BF], F32, tag="esum", name="esc")
        nc.vector.tensor_tensor(esc[:], Ec0[:], Ec1[:], op=OP.add)
        recc = sm_pool.tile([16, BF], F32, tag="rec", name="recc")
        nc.vector.reciprocal_approx_fast(recc[:], esc[:])
        Pc0 = pp_pool.tile([16, BF], BF16, tag="P0", name="Pc0")
        nc.vector.tensor_tensor(Pc0[:], Ec0[:], recc[:], op=OP.mult)
        Pc1 = pp_pool.tile([16, BF], BF16, tag="P1", name="Pc1")
        nc.vector.tensor_tensor(Pc1[:], Ec1[:], recc[:], op=OP.mult)
        # cv for img/aud: Wkv og 4..7 (cols 1024..2047)
        cvs = {}
        for og in range(4, 8):
            wt = wog_pool.tile([128, NCH * 256], BF16, tag="wog", name="w_wv_og")
            wsl = ins["wkv_p"][:, og * NCH * 256:(og + 1) * NCH * 256]
            nc.sync.dma_start(wt[:], wsl)
            wtv = wt[:].rearrange("p (k n) -> p k n", k=NCH)
            for tok in (1, 2):
                for jj in range(2):
                    c = (og - 4) * 2 + jj
                    ps = ps_big.tile([128, BF], F32, tag="big")
                    for k in range(NCH):
                        nc.tensor.matmul(ps[:], wtv[:, k, jj * 128:(jj + 1) * 128],
                                         xs[(tok, k)][:], start=(k == 0), stop=(k == NCH - 1))
                    cv = cv_pool.tile([128, BF], BF16, tag=f"cv_{tok}_{c}")
                    nc.scalar.copy(cv[:], ps[:])
                    cvs[(tok, c)] = cv
        # weighted cv sum -> cross attnout
        caos = {}
        for c in range(NCH):
            pe_i = ps_big.tile([128, BF], F32, tag="big")
            nc.tensor.matmul(pe_i[:], selb_c(c), Pc0[:], start=True, stop=True)
            pe_a = ps_big.tile([128, BF], F32, tag="big")
            nc.tensor.matmul(pe_a[:], selb_c(c), Pc1[:], start=True, stop=True)
            t0 = tt_pool.tile([128, BF], BF16, tag="tt")
            nc.vector.tensor_tensor(t0[:], pe_i[:], cvs[(1, c)][:], op=OP.mult)
            t1 = tt_pool.tile([128, BF], BF16, tag="tt")
            nc.vector.tensor_tensor(t1[:], pe_a[:], cvs[(2, c)][:], op=OP.mult)
            cao = ao_pool.tile([128, BF], BF16, tag=f"ao_{c}", name=f"cao_{c}")
            nc.vector.tensor_tensor(cao[:], t0[:], t1[:], op=OP.add)
            caos[c] = cao
        # ca_proj + bias + residual (in-place into text xs)
        for og in range(4):
            wt = wog_pool.tile([128, NCH * 256], BF16, tag="wog", name="w_wca_og")
            wsl = ins["wca_p"][:, og * NCH * 256:(og + 1) * NCH * 256]
            nc.sync.dma_start(wt[:], wsl)
            wtv = wt[:].rearrange("p (k n) -> p k n", k=NCH)
            for jj in range(2):
                o = og * 2 + jj
                ps = ps_big.tile([128, BF], F32, tag="big")
                for k in range(NCH):
                    nc.tensor.matmul(ps[:], wtv[:, k, jj * 128:(jj + 1) * 128],
                                     caos[k][:], start=(k == 0), stop=(k == NCH - 1))
                nc.vector.scalar_tensor_tensor(
                    xs[(0, o)][:], ps[:], cols["cab"][:, o:o + 1],
                    xs[(0, o)][:], op0=OP.add, op1=OP.add)

        # ---- phase 7: LN2 on text token, f32 out, store ----
        outs_t = [out_pool.tile([128, BF], F32, tag=f"out_{c % 2}", name=f"out_{c}")
                  for c in range(NCH)]
        ln_group([[xs[(0, c)][:] for c in range(NCH)]],
                 cols["l2g"], cols["l2b"], [[o[:] for o in outs_t]])
        for c in range(NCH):
            nc.sync.dma_start(out_t[c * 128:(c + 1) * 128, bs:bs + BF], outs_t[c][:])

    for blk in range(nblk):
        tail(head(blk))

    stack.close()


# ------------------------------------------------------------------ host side

def _prep_shared(Wqkv, sa_proj_w, sa_proj_b, ln1_g, ln1_b, Wq, Wkv, ca_proj_w,
                 ca_proj_b, ln2_g, ln2_b):
    f = np.float32
    bf = ml_dtypes.bfloat16

    def kperm(W):  # [1024, N] -> [128, og, 8k, 256] flat (og-contiguous slabs)
        N = W.shape[1]
        kp = W.reshape(NCH, 128, N).transpose(1, 0, 2)          # [128, 8k, N]
        nog = N // 256
        og = kp.reshape(128, NCH, nog, 256).transpose(0, 2, 1, 3)  # [128, og, k, 256]
        return np.ascontiguousarray(og.reshape(128, N * NCH)).astype(bf)

    # Wqkv: per out-chunk c: [q_c | k_c | v_c] columns adjacent; SCALE folded
    # into the q block (so scores need no extra scaling).
    Wq3 = np.asarray(Wqkv, f).reshape(1024, 3, NCH, 128).copy()  # [k, qkv, c, 128]
    Wq3[:, 0] *= SCALE
    per_c = []
    for c in range(NCH):
        colsq = np.concatenate([Wq3[:, t, c, :] for t in range(3)], axis=1)  # [1024,384]
        per_c.append(colsq.reshape(NCH, 128, 384).transpose(1, 0, 2).reshape(128, NCH * 384))
    wqkv_p = np.ascontiguousarray(np.concatenate(per_c, axis=1)).astype(bf)

    sel = np.zeros((128, NCH, 16), f)
    for r in range(128):
        for c in range(NCH):
            sel[r, c, 2 * c + r // 64] = 1.0
    selb1 = np.zeros((16, NCH, 128), f)
    for h in range(16):
        for c in range(NCH):
            for m in range(128):
                if h == 2 * c + m // 64:
                    selb1[h, c, m] = 1.0
    selb = np.zeros((96, NCH, 128), f)
    for b0 in (0, 32, 64):
        selb[b0:b0 + 16] = selb1
    onesrow3 = np.zeros((65, 128), f)
    for b0 in (0, 32, 64):
        onesrow3[b0] = 1.0
    col = lambda v: np.ascontiguousarray(np.asarray(v, f).reshape(NCH, 128).T)
    return {
        "wqkv_p": wqkv_p,
        "wsa_p": kperm(np.asarray(sa_proj_w, f)),
        "wq_p": kperm(np.asarray(Wq, f) * SCALE),
        "wkv_p": kperm(np.asarray(Wkv, f)),
        "wca_p": kperm(np.asarray(ca_proj_w, f)),
        "sel": sel.reshape(128, NCH * 16).astype(bf),
        "selb": selb.reshape(96, NCH * 128).astype(bf),
        "ones_col": np.full((128, 1), 1.0 / D, f).astype(bf),
        "ones_row": onesrow3.astype(bf),
        "sab": col(sa_proj_b), "l1g": col(ln1_g), "l1b": col(ln1_b),
        "cab": col(ca_proj_b), "l2g": col(ln2_g), "l2b": col(ln2_b),
    }


_CACHE = {}


def _pin_act_tables(arch):
    """Force Exp and Ln to resolve to the one act-table set that holds both
    (natural_log_exp_and_others), so the kernel needs a single table load
    instead of thrashing between the exp- and ln-only sets. Mutates the
    cached membership sets in place; set ids/order are untouched."""
    from concourse import hw_specs
    tabs = hw_specs.get_activation_tables(arch)
    if "natural_log_exp_and_others" not in tabs:
        return
    for name, s in tabs.items():
        if name != "natural_log_exp_and_others":
            s.discard(AF.Exp)
            s.discard(AF.Ln)


def _get_program(nblk):
    if nblk in _CACHE:
        return _CACHE[nblk]
    nc = bacc.Bacc("TRN2", target_bir_lowering=False, debug=False,
                   enable_asserts=False, num_devices=NCORES)
    # note: pinning Exp/Ln into one act-table set (see _pin_act_tables) removes
    # the per-block ACT_TABLE_LOADs but perturbs the tile scheduler's plan for
    # a net loss (-91us measured) — intentionally NOT applied.
    ins = {}
    bl = nblk * BF
    for nm in ("xt_t", "xt_i", "xt_a"):
        ins[nm] = nc.dram_tensor(nm, [D, bl], BF16, kind="ExternalInput").ap()
    ins["wqkv_p"] = nc.dram_tensor("wqkv_p", [128, NCH * NCH * 384], BF16, kind="ExternalInput").ap()
    for nm, w in (("wsa_p", 1024), ("wq_p", 1024), ("wkv_p", 2048), ("wca_p", 1024)):
        ins[nm] = nc.dram_tensor(nm, [128, NCH * w], BF16, kind="ExternalInput").ap()
    ins["sel"] = nc.dram_tensor("sel", [128, NCH * 16], BF16, kind="ExternalInput").ap()
    ins["selb"] = nc.dram_tensor("selb", [96, NCH * 128], BF16, kind="ExternalInput").ap()
    ins["ones_col"] = nc.dram_tensor("ones_col", [128, 1], BF16, kind="ExternalInput").ap()
    ins["ones_row"] = nc.dram_tensor("ones_row", [65, 128], BF16, kind="ExternalInput").ap()
    for nm in ("sab", "l1g", "l1b", "cab", "l2g", "l2b"):
        ins[nm] = nc.dram_tensor(nm, [128, NCH], F32, kind="ExternalInput").ap()
    outs = {"out_t": nc.dram_tensor("out_t", [D, bl], F32, kind="ExternalOutput").ap()}

    with tile.TileContext(nc) as tc:
        build(tc, outs, ins, nblk)
    nc.compile()
    _CACHE[nblk] = nc
    return nc


def kernel(c_text, c_image, c_audio, Wqkv, sa_proj_w, sa_proj_b, ln1_g, ln1_b,
           Wq, Wkv, ca_proj_w, ca_proj_b, ln2_g, ln2_b, _trace=False):
    bf = ml_dtypes.bfloat16
    shared = _prep_shared(Wqkv, sa_proj_w, sa_proj_b, ln1_g, ln1_b, Wq, Wkv,
                          ca_proj_w, ca_proj_b, ln2_g, ln2_b)
    xT = {
        "xt_t": np.ascontiguousarray(np.asarray(c_text, np.float32).T).astype(bf),
        "xt_i": np.ascontiguousarray(np.asarray(c_image, np.float32).T).astype(bf),
        "xt_a": np.ascontiguousarray(np.asarray(c_audio, np.float32).T).astype(bf),
    }
    in_maps = []
    for s in range(NCORES):
        sl = slice(s * BLOC, (s + 1) * BLOC)
        m = dict(shared)
        for k in xT:
            m[k] = np.ascontiguousarray(xT[k][:, sl])
        in_maps.append(m)
    nc = _get_program(NBLK_HW)
    res = bass_utils.run_bass_kernel_spmd(nc, in_maps, core_ids=list(range(NCORES)),
                                          trace=_trace)
    out = np.concatenate([np.asarray(r["out_t"]).T for r in res.results], axis=0)
    if _trace:
        kernel.last_results = res
    return out.astype(np.float32)


# revision 46
# speedup vs baseline: 1.1904x; 1.0002x over previous
"""Trainium2 Bass kernel for nn_MCFL_49254684950998 (dense multimodal transformer block).

Strategy: pure data parallel over 8 NeuronCores (batch 16384 -> 2048/core).
Feature-major layout ("T layout": [feat_chunk=128, batch]); host pre-transposes
inputs and post-transposes outputs. All GEMMs run in bf16 (full PE rate,
fp32 PSUM accumulate). Attention (3-token self-attn + 2-token cross-attn)
uses DVE bf16 elementwise products + selection matmuls on the TensorEngine;
softmax and LayerNorm reciprocals via ACT ln/exp (one act table, no swaps);
LayerNorm stats via col-packed ones-matmuls, apply via rank-1 PE broadcasts
plus fused tensor_scalar. Buffers are pool-allocated with bufs>=2 on the
block-critical paths so the Tile scheduler overlaps adjacent batch blocks
(keeps PE warm / HAM un-throttled).
"""

import sys

sys.path.insert(0, "/opt/trn_rl_repo")

import numpy as np
import ml_dtypes

import concourse.bass as bass
import concourse.bacc as bacc
import concourse.tile as tile
import concourse.mybir as mybir
from concourse import bass_utils

F32 = mybir.dt.float32
BF16 = mybir.dt.bfloat16
AF = mybir.ActivationFunctionType
OP = mybir.AluOpType

B, D, H, HD = 16384, 1024, 16, 64
NCORES = 8
BLOC = B // NCORES          # 2048 batch rows per core
BF = 512                    # batch tile (free dim) per block
NBLK_HW = BLOC // BF        # 4 blocks per core
NCH = D // 128              # 8 feature chunks
SCALE = HD ** -0.5
EPS = 1e-5


def build(tc, outs, ins, nblk):
    from contextlib import ExitStack
    stack = ExitStack()
    nc = tc.nc
    out_t = outs["out_t"]
    xt = [ins["xt_t"], ins["xt_i"], ins["xt_a"]]

    # ---- const tiles (loaded once) ----
    cpool = stack.enter_context(tc.tile_pool(name="consts", bufs=1))
    sel_sb = cpool.tile([128, NCH * 16], BF16, tag="sel")       # [128, c, 16]
    nc.sync.dma_start(sel_sb[:], ins["sel"][:])
    selb_sb = cpool.tile([96, NCH * 128], BF16, tag="selb")     # bases 0/32/64
    nc.sync.dma_start(selb_sb[:], ins["selb"][:])
    ones_sb = cpool.tile([128, 1], BF16, tag="ones")            # 1/1024
    nc.sync.dma_start(ones_sb[:], ins["ones_col"][:])
    onesrow_sb = cpool.tile([65, 128], BF16, tag="onesrow")     # 1.0 @ rows 0/32/64
    nc.sync.dma_start(onesrow_sb[:], ins["ones_row"][:])
    cols = {}
    for nm in ("sab", "l1g", "l1b", "cab", "l2g", "l2b"):
        cols[nm] = cpool.tile([128, NCH], F32, tag=nm, name=f"col_{nm}")
        nc.sync.dma_start(cols[nm][:], ins[nm][:])

    def sel_c(c):
        return sel_sb[:, c * 16:(c + 1) * 16]

    def selb_c(c, base=0):
        return selb_sb[base:base + 16, c * 128:(c + 1) * 128]

    # ---- pools ----
    def pool(*a, **k):
        return stack.enter_context(tc.tile_pool(*a, **k))

    wq_pool = pool(name="wqkv", bufs=3)       # [128, 8*384] bf16 (6KB)
    wog_pool = pool(name="wog", bufs=3)       # [128, 8*256] bf16 (4KB)
    x_pool = pool(name="xs", bufs=2)          # 24 tags [128,BF] bf16
    qk_pool = pool(name="qk", bufs=8)         # [128, BF] bf16
    v_pool = pool(name="vs", bufs=1)          # 24 tags [128,BF] bf16
    pr_pool = pool(name="prod", bufs=4)       # [128, BF] bf16
    tt_pool = pool(name="tt", bufs=4)         # [128, BF] bf16 (attnout terms)
    ao_pool = pool(name="ao", bufs=2)         # 8 tags [128,BF] bf16
    cq_pool = pool(name="cq", bufs=1)         # 8 tags [128, BF] bf16
    ck_pool = pool(name="ck", bufs=4)         # [128, BF] bf16
    cv_pool = pool(name="cv", bufs=1)         # 16 tags [128,BF] bf16
    sq_pool = pool(name="sq", bufs=2)         # [128, BF] bf16 squares
    sm_pool = pool(name="sm", bufs=1)         # small f32/bf16 softmax+LN tiles
    pp_pool = pool(name="pp", bufs=2)         # softmax P tiles (cross-block)
    out_pool = pool(name="outp", bufs=1)      # 2 tags [128,BF] f32
    rbc_pool = pool(name="rbc", bufs=2)       # [128,BF] bf16 rstd broadcast
    ps_big = pool(name="psbig", bufs=5, space="PSUM")
    ps_S = pool(name="psS", bufs=3, space="PSUM")

    def ln_group(ylists, gcol, bcol, dstlists):
        """Layernorm over the feature (partition-chunk) dim for 1-3 tokens at
        once. ylists/dstlists: per-token lists of 8 [128,BF] APs. Token t's
        stats live at partition 32t of two PSUM banks (col-group packing);
        smalls processed on rows [0:R] in one shot (junk rows harmless)."""
        ntok = len(ylists)
        R = 32 * (ntok - 1) + 1
        stA = ps_S.tile([128, BF], F32, tag="S", name="stA")
        stB = ps_S.tile([128, BF], F32, tag="S", name="stB")
        for t in range(ntok):
            b0 = 32 * t
            for c in range(NCH):
                sq = sq_pool.tile([128, BF], BF16, tag="sq")
                nc.gpsimd.tensor_tensor(sq[:], ylists[t][c], ylists[t][c],
                                        op=OP.mult)
                st, sp = (c == 0), (c == NCH - 1)
                nc.tensor.matmul(stA[b0:b0 + 1, :], ones_sb[:], ylists[t][c],
                                 start=st, stop=sp, tile_position=(0, b0),
                                 skip_group_check=True)
                nc.tensor.matmul(stB[b0:b0 + 1, :], ones_sb[:], sq[:],
                                 start=st, stop=sp, tile_position=(0, b0),
                                 skip_group_check=True)
        mu_s = sm_pool.tile([65, BF], F32, tag="mu_s")
        nc.vector.tensor_copy(mu_s[0:R, :], stA[0:R, :])
        mu2 = sm_pool.tile([65, BF], F32, tag="lnvr", name="mu2")
        nc.vector.tensor_tensor(mu2[0:R, :], mu_s[0:R, :], mu_s[0:R, :], op=OP.mult)
        var = sm_pool.tile([65, BF], F32, tag="var")
        nc.vector.scalar_tensor_tensor(var[0:R, :], stB[0:R, :], EPS, mu2[0:R, :],
                                       op0=OP.add, op1=OP.subtract)
        # keep-warm matmuls chained into the smalls serial chain: the PE would
        # otherwise idle >3.4us here and HAM-rethrottle (writes land in unused
        # rows of the stats bank; values are garbage and never read)
        nc.tensor.matmul(stA[96:112, 0:64], sel_sb[0:R, 0:16],
                         var[:].bitcast(BF16)[0:R, 0:64], start=True, stop=True,
                         tile_position=(0, 96), skip_group_check=True)
        lnv = sm_pool.tile([65, BF], F32, tag="lnvr")
        nc.scalar.activation(lnv[0:R, :], var[0:R, :], AF.Ln)
        rstd = sm_pool.tile([65, BF], BF16, tag="rstd")
        nc.scalar.activation(rstd[0:R, :], lnv[0:R, :], AF.Exp, scale=-0.5)
        nc.tensor.matmul(stA[96:112, 0:64], sel_sb[0:R, 0:16],
                         rstd[0:R, 0:64], start=True, stop=True,
                         tile_position=(0, 96), skip_group_check=True)
        mup = sm_pool.tile([65, BF], BF16, tag="mup")
        nc.vector.tensor_tensor(mup[0:R, :], mu_s[0:R, :], rstd[0:R, :], op=OP.mult)
        for t in range(ntok):
            b0 = 32 * t
            rb_ps = ps_S.tile([128, BF], F32, tag="S", name="rb_ps")
            nc.tensor.matmul(rb_ps[:], onesrow_sb[b0:b0 + 1, :],
                             rstd[b0:b0 + 1, :], start=True, stop=True)
            rb = rbc_pool.tile([128, BF], BF16, tag="rbc")
            nc.scalar.copy(rb[:], rb_ps[:])
            mu_ps = ps_S.tile([128, BF], F32, tag="S", name="mu_ps")
            nc.tensor.matmul(mu_ps[:], onesrow_sb[b0:b0 + 1, :],
                             mup[b0:b0 + 1, :], start=True, stop=True)
            for c in range(NCH):
                t1 = tt_pool.tile([128, BF], BF16, tag="tt")
                nc.vector.tensor_tensor(t1[:], ylists[t][c], rb[:], op=OP.mult)
                t2 = tt_pool.tile([128, BF], BF16, tag="tt")
                nc.vector.tensor_tensor(t2[:], t1[:], mu_ps[:], op=OP.subtract)
                nc.vector.tensor_scalar(dstlists[t][c], t2[:], gcol[:, c:c + 1],
                                        bcol[:, c:c + 1], op0=OP.mult, op1=OP.add)

    def head(blk):
        """Phases 1-3: x load, qkv GEMM + scores, softmax -> P tiles."""
        bs = blk * BF
        # first weight slab before the x queue so the PE isn't DMA-gated
        wt0 = wq_pool.tile([128, NCH * 384], BF16, tag="wqkv", name="wt0")
        wsl0 = ins["wqkv_p"][:, 0:NCH * 384]
        half = NCH * 192
        nc.sync.dma_start(wt0[:, :half], wsl0[:, :half])
        nc.scalar.dma_start(wt0[:, half:], wsl0[:, half:])
        xs = {}
        qs = (nc.sync, nc.scalar)
        for m in range(3):
            for c in range(NCH):
                t = x_pool.tile([128, BF], BF16, tag=f"x_{m}_{c}")
                qs[(m * NCH + c) % 2].dma_start(
                    t[:], xt[m][c * 128:(c + 1) * 128, bs:bs + BF])
                xs[(m, c)] = t

        S_banks = [ps_S.tile([128, BF], F32, tag="S", name=f"Sbank{i}")
                   for i in range(3)]
        vs = {}

        def emit_products(c, qts, kts):
            for i in range(3):
                for j in range(3):
                    pr = pr_pool.tile([128, BF], BF16, tag="prod")
                    nc.vector.tensor_tensor(pr[:], qts[i][:], kts[j][:], op=OP.mult)
                    nc.tensor.matmul(
                        S_banks[j][32 * i:32 * i + 16, :],
                        sel_c(c), pr[:],
                        start=(c == 0), stop=(c == NCH - 1),
                        tile_position=(0, 32 * i),
                        skip_group_check=True,
                    )

        for c in range(NCH):
            if c == 0:
                wt = wt0
            else:
                wt = wq_pool.tile([128, NCH * 384], BF16, tag="wqkv")
                wsl = ins["wqkv_p"][:, c * NCH * 384:(c + 1) * NCH * 384]
                nc.sync.dma_start(wt[:, :half], wsl[:, :half])
                nc.scalar.dma_start(wt[:, half:], wsl[:, half:])
            qts, kts = [], []
            for m in range(3):
                psq = ps_big.tile([128, BF], F32, tag="big")
                psk = ps_big.tile([128, BF], F32, tag="big")
                psv = ps_big.tile([128, BF], F32, tag="big")
                for k in range(NCH):
                    wk = wt[:, k * 384:(k + 1) * 384]
                    st, sp = (k == 0), (k == NCH - 1)
                    nc.tensor.matmul(psq[:], wk[:, 0:128], xs[(m, k)][:], start=st, stop=sp)
                    nc.tensor.matmul(psk[:], wk[:, 128:256], xs[(m, k)][:], start=st, stop=sp)
                    nc.tensor.matmul(psv[:], wk[:, 256:384], xs[(m, k)][:], start=st, stop=sp)
                qt = qk_pool.tile([128, BF], BF16, tag="qk")
                kt = qk_pool.tile([128, BF], BF16, tag="qk")
                vt = v_pool.tile([128, BF], BF16, tag=f"v_{m}_{c}")
                nc.scalar.copy(qt[:], psq[:])
                nc.scalar.copy(kt[:], psk[:])
                nc.scalar.copy(vt[:], psv[:])
                qts.append(qt)
                kts.append(kt)
                vs[(m, c)] = vt
            emit_products(c, qts, kts)

        def warm(src):
            """Tiny matmul chained on `src` ([80,BF] tile) into unused rows of
            S bank 0 — keeps the PE HAM window busy through the softmax
            serial chain (a >3.4us PE-idle gap would re-throttle the clock).
            Values are garbage (bitcast) — the output rows are never read."""
            mv = src[:].bitcast(BF16) if src.dtype == F32 else src[:]
            nc.tensor.matmul(S_banks[0][96:112, 0:64], sel_sb[0:80, 0:16],
                             mv[0:80, 0:64], start=True, stop=True,
                             tile_position=(0, 96), skip_group_check=True)

        # softmax over 3 keys; tokens packed at partition bases 0/32/64
        E = []
        for j in range(3):
            e = sm_pool.tile([80, BF], F32, tag=f"E{j}", name=f"E{j}")
            nc.scalar.activation(e[:], S_banks[j][0:80, :], AF.Exp)
            E.append(e)
        warm(E[0])
        esum = sm_pool.tile([80, BF], F32, tag="esum")
        nc.vector.tensor_tensor(esum[:], E[0][:], E[1][:], op=OP.add)
        nc.vector.tensor_tensor(esum[:], esum[:], E[2][:], op=OP.add)
        warm(esum)
        rec = sm_pool.tile([80, BF], F32, tag="rec")
        nc.vector.reciprocal_approx_fast(rec[:], esum[:])
        warm(rec)
        P = []
        for j in range(3):
            p = pp_pool.tile([80, BF], BF16, tag=f"P{j}", name=f"P{j}")
            nc.vector.tensor_tensor(p[:], E[j][:], rec[:], op=OP.mult)
            P.append(p)
        return dict(bs=bs, xs=xs, vs=vs, P=P)

    def tail(st):
        """Phases 4-7 for a block whose head already ran."""
        bs, xs, vs, P = st["bs"], st["xs"], st["vs"], st["P"]
        # ---- phase 4: per token: weighted sum, sa_proj, residual ----
        for tok in range(3):
            b0 = 32 * tok
            aos = {}
            for c in range(NCH):
                ts = []
                for j in range(3):
                    pe = ps_big.tile([128, BF], F32, tag="big")
                    nc.tensor.matmul(pe[:], selb_c(c, b0),
                                     P[j][b0:b0 + 16, :], start=True, stop=True)
                    t = tt_pool.tile([128, BF], BF16, tag="tt")
                    nc.vector.tensor_tensor(t[:], pe[:], vs[(j, c)][:], op=OP.mult)
                    ts.append(t)
                ao = ao_pool.tile([128, BF], BF16, tag=f"ao_{c}")
                nc.vector.tensor_tensor(ao[:], ts[0][:], ts[1][:], op=OP.add)
                nc.vector.tensor_tensor(ao[:], ao[:], ts[2][:], op=OP.add)
                aos[c] = ao
            # sa_proj for this token + bias + residual (in-place into xs)
            for og in range(4):
                wt = wog_pool.tile([128, NCH * 256], BF16, tag="wog",
                                   name=f"wsa_{tok}_{og}")
                wsl = ins["wsa_p"][:, og * NCH * 256:(og + 1) * NCH * 256]
                nc.sync.dma_start(wt[:], wsl)
                wtv = wt[:].rearrange("p (k n) -> p k n", k=NCH)
                for jj in range(2):
                    o = og * 2 + jj
                    ps = ps_big.tile([128, BF], F32, tag="big")
                    for k in range(NCH):
                        nc.tensor.matmul(ps[:], wtv[:, k, jj * 128:(jj + 1) * 128],
                                         aos[k][:], start=(k == 0), stop=(k == NCH - 1))
                    nc.vector.scalar_tensor_tensor(
                        xs[(tok, o)][:], ps[:], cols["sab"][:, o:o + 1],
                        xs[(tok, o)][:], op0=OP.add, op1=OP.add)

        # ---- phase 5: LN1, 3 tokens packed ----
        ln_group([[xs[(tok, c)][:] for c in range(NCH)] for tok in range(3)],
                 cols["l1g"], cols["l1b"],
                 [[xs[(tok, c)][:] for c in range(NCH)] for tok in range(3)])

        # ---- phase 6: cross attention ----
        # cq = t_text @ Wq (scale folded host-side)
        cqs = {}
        for og in range(4):
            wt = wog_pool.tile([128, NCH * 256], BF16, tag="wog", name="w_wq_og")
            wsl = ins["wq_p"][:, og * NCH * 256:(og + 1) * NCH * 256]
            nc.sync.dma_start(wt[:], wsl)
            wtv = wt[:].rearrange("p (k n) -> p k n", k=NCH)
            for jj in range(2):
                o = og * 2 + jj
                ps = ps_big.tile([128, BF], F32, tag="big")
                for k in range(NCH):
                    nc.tensor.matmul(ps[:], wtv[:, k, jj * 128:(jj + 1) * 128],
                                     xs[(0, k)][:], start=(k == 0), stop=(k == NCH - 1))
                cq = cq_pool.tile([128, BF], BF16, tag=f"cq_{o}")
                nc.scalar.copy(cq[:], ps[:])
                cqs[o] = cq
        # ck for img(tok1), aud(tok2): Wkv og 0..3 (cols 0..1023)
        Sc0 = ps_S.tile([128, BF], F32, tag="S", name="Sc0")
        Sc1 = ps_S.tile([128, BF], F32, tag="S", name="Sc1")
        Scs = [Sc0, Sc1]
        n_seen = [0, 0]  # per jj-pair chunk counter for S accumulation
        for og in range(4):
            wt = wog_pool.tile([128, NCH * 256], BF16, tag="wog", name="w_wk_og")
            wsl = ins["wkv_p"][:, og * NCH * 256:(og + 1) * NCH * 256]
            nc.sync.dma_start(wt[:], wsl)
            wtv = wt[:].rearrange("p (k n) -> p k n", k=NCH)
            for tok in (1, 2):
                for jj in range(2):
                    c = og * 2 + jj
                    ps = ps_big.tile([128, BF], F32, tag="big")
                    for k in range(NCH):
                        nc.tensor.matmul(ps[:], wtv[:, k, jj * 128:(jj + 1) * 128],
                                         xs[(tok, k)][:], start=(k == 0), stop=(k == NCH - 1))
                    ck = ck_pool.tile([128, BF], BF16, tag="ck")
                    nc.scalar.copy(ck[:], ps[:])
                    pi = tok - 1  # 0 = img, 1 = aud
                    pr = pr_pool.tile([128, BF], BF16, tag="prod")
                    nc.vector.tensor_tensor(pr[:], cqs[c][:], ck[:], op=OP.mult)
                    nc.tensor.matmul(
                        Scs[pi][0:16, :], sel_c(c), pr[:],
                        start=(n_seen[pi] == 0), stop=(n_seen[pi] == NCH - 1))
                    n_seen[pi] += 1
        # cross softmax over 2 keys (tags shared with self-softmax tiles,
        # lifetimes are disjoint within a block)
        Ec0 = sm_pool.tile([16, BF], F32, tag="E0", name="Ec0")
        nc.scalar.activation(Ec0[:], Sc0[0:16, :], AF.Exp)
        Ec1 = sm_pool.tile([16, BF], F32, tag="E1", name="Ec1")
        nc.scalar.activation(Ec1[:], Sc1[0:16, :], AF.Exp)
        esc = sm_pool.tile([16, BF], F32, tag="esum", name="esc")
        nc.vector.tensor_tensor(esc[:], Ec0[:], Ec1[:], op=OP.add)
        recc = sm_pool.tile([16, BF], F32, tag="rec", name="recc")
        nc.vector.reciprocal_approx_fast(recc[:], esc[:])
        Pc0 = pp_pool.tile([16, BF], BF16, tag="P0", name="Pc0")
        nc.vector.tensor_tensor(Pc0[:], Ec0[:], recc[:], op=OP.mult)
        Pc1 = pp_pool.tile([16, BF], BF16, tag="P1", name="Pc1")
        nc.vector.tensor_tensor(Pc1[:], Ec1[:], recc[:], op=OP.mult)
        # cv for img/aud: Wkv og 4..7 (cols 1024..2047)
        cvs = {}
        for og in range(4, 8):
            wt = wog_pool.tile([128, NCH * 256], BF16, tag="wog", name="w_wv_og")
            wsl = ins["wkv_p"][:, og * NCH * 256:(og + 1) * NCH * 256]
            nc.sync.dma_start(wt[:], wsl)
            wtv = wt[:].rearrange("p (k n) -> p k n", k=NCH)
            for tok in (1, 2):
                for jj in range(2):
                    c = (og - 4) * 2 + jj
                    ps = ps_big.tile([128, BF], F32, tag="big")
                    for k in range(NCH):
                        nc.tensor.matmul(ps[:], wtv[:, k, jj * 128:(jj + 1) * 128],
                                         xs[(tok, k)][:], start=(k == 0), stop=(k == NCH - 1))
                    cv = cv_pool.tile([128, BF], BF16, tag=f"cv_{tok}_{c}")
                    nc.scalar.copy(cv[:], ps[:])
                    cvs[(tok, c)] = cv
        # weighted cv sum -> cross attnout
        caos = {}
        for c in range(NCH):
            pe_i = ps_big.tile([128, BF], F32, tag="big")
            nc.tensor.matmul(pe_i[:], selb_c(c), Pc0[:], start=True, stop=True)
            pe_a = ps_big.tile([128, BF], F32, tag="big")
            nc.tensor.matmul(pe_a[:], selb_c(c), Pc1[:], start=True, stop=True)
            t0 = tt_pool.tile([128, BF], BF16, tag="tt")
            nc.vector.tensor_tensor(t0[:], pe_i[:], cvs[(1, c)][:], op=OP.mult)
            t1 = tt_pool.tile([128, BF], BF16, tag="tt")
            nc.vector.tensor_tensor(t1[:], pe_a[:], cvs[(2, c)][:], op=OP.mult)
            cao = ao_pool.tile([128, BF], BF16, tag=f"ao_{c}", name=f"cao_{c}")
            nc.vector.tensor_tensor(cao[:], t0[:], t1[:], op=OP.add)
            caos[c] = cao
        # ca_proj + bias + residual (in-place into text xs)
        for og in range(4):
            wt = wog_pool.tile([128, NCH * 256], BF16, tag="wog", name="w_wca_og")
            wsl = ins["wca_p"][:, og * NCH * 256:(og + 1) * NCH * 256]
            nc.sync.dma_start(wt[:], wsl)
            wtv = wt[:].rearrange("p (k n) -> p k n", k=NCH)
            for jj in range(2):
                o = og * 2 + jj
                ps = ps_big.tile([128, BF], F32, tag="big")
                for k in range(NCH):
                    nc.tensor.matmul(ps[:], wtv[:, k, jj * 128:(jj + 1) * 128],
                                     caos[k][:], start=(k == 0), stop=(k == NCH - 1))
                nc.vector.scalar_tensor_tensor(
                    xs[(0, o)][:], ps[:], cols["cab"][:, o:o + 1],
                    xs[(0, o)][:], op0=OP.add, op1=OP.add)

        # ---- phase 7: LN2 on text token, f32 out, store ----
        outs_t = [out_pool.tile([128, BF], F32, tag=f"out_{c % 2}", name=f"out_{c}")
                  for c in range(NCH)]
        ln_group([[xs[(0, c)][:] for c in range(NCH)]],
                 cols["l2g"], cols["l2b"], [[o[:] for o in outs_t]])
        for c in range(NCH):
            nc.sync.dma_start(out_t[c * 128:(c + 1) * 128, bs:bs + BF], outs_t[c][:])

    for blk in range(nblk):
        tail(head(blk))

    stack.close()


# ------------------------------------------------------------------ host side

def _prep_shared(Wqkv, sa_proj_w, sa_proj_b, ln1_g, ln1_b, Wq, Wkv, ca_proj_w,
                 ca_proj_b, ln2_g, ln2_b):
    f = np.float32
    bf = ml_dtypes.bfloat16

    def kperm(W):  # [1024, N] -> [128, og, 8k, 256] flat (og-contiguous slabs)
        N = W.shape[1]
        kp = W.reshape(NCH, 128, N).transpose(1, 0, 2)          # [128, 8k, N]
        nog = N // 256
        og = kp.reshape(128, NCH, nog, 256).transpose(0, 2, 1, 3)  # [128, og, k, 256]
        return np.ascontiguousarray(og.reshape(128, N * NCH)).astype(bf)

    # Wqkv: per out-chunk c: [q_c | k_c | v_c] columns adjacent; SCALE folded
    # into the q block (so scores need no extra scaling).
    Wq3 = np.asarray(Wqkv, f).reshape(1024, 3, NCH, 128).copy()  # [k, qkv, c, 128]
    Wq3[:, 0] *= SCALE
    per_c = []
    for c in range(NCH):
        colsq = np.concatenate([Wq3[:, t, c, :] for t in range(3)], axis=1)  # [1024,384]
        per_c.append(colsq.reshape(NCH, 128, 384).transpose(1, 0, 2).reshape(128, NCH * 384))
    wqkv_p = np.ascontiguousarray(np.concatenate(per_c, axis=1)).astype(bf)

    sel = np.zeros((128, NCH, 16), f)
    for r in range(128):
        for c in range(NCH):
            sel[r, c, 2 * c + r // 64] = 1.0
    selb1 = np.zeros((16, NCH, 128), f)
    for h in range(16):
        for c in range(NCH):
            for m in range(128):
                if h == 2 * c + m // 64:
                    selb1[h, c, m] = 1.0
    selb = np.zeros((96, NCH, 128), f)
    for b0 in (0, 32, 64):
        selb[b0:b0 + 16] = selb1
    onesrow3 = np.zeros((65, 128), f)
    for b0 in (0, 32, 64):
        onesrow3[b0] = 1.0
    col = lambda v: np.ascontiguousarray(np.asarray(v, f).reshape(NCH, 128).T)
    return {
        "wqkv_p": wqkv_p,
        "wsa_p": kperm(np.asarray(sa_proj_w, f)),
        "wq_p": kperm(np.asarray(Wq, f) * SCALE),
        "wkv_p": kperm(np.asarray(Wkv, f)),
        "wca_p": kperm(np.asarray(ca_proj_w, f)),
        "sel": sel.reshape(128, NCH * 16).astype(bf),
        "selb": selb.reshape(96, NCH * 128).astype(bf),
        "ones_col": np.full((128, 1), 1.0 / D, f).astype(bf),
        "ones_row": onesrow3.astype(bf),
        "sab": col(sa_proj_b), "l1g": col(ln1_g), "l1b": col(ln1_b),
        "cab": col(ca_proj_b), "l2g": col(ln2_g), "l2b": col(ln2_b),
    }


_CACHE = {}


def _pin_act_tables(arch):
    """Force Exp and Ln to resolve to the one act-table set that holds both
    (natural_log_exp_and_others), so the kernel needs a single table load
    instead of thrashing between the exp- and ln-only sets. Mutates the
    cached membership sets in place; set ids/order are untouched."""
    from concourse import hw_specs
    tabs = hw_specs.get_activation_tables(arch)
    if "natural_log_exp_and_others" not in tabs:
        return
    for name, s in tabs.items():
        if name != "natural_log_exp_and_others":
            s.discard(AF.Exp)
            s.discard(AF.Ln)


def _get_program(nblk):
    if nblk in _CACHE:
        return _CACHE[nblk]
    nc = bacc.Bacc("TRN2", target_bir_lowering=False, debug=False,
                   enable_asserts=False, num_devices=NCORES)
    # note: pinning Exp/Ln into one act-table set (see _pin_act_tables) removes
    # the per-block ACT_TABLE_LOADs but perturbs the tile scheduler's plan for
    # a net loss (-91us measured) — intentionally NOT applied.
    ins = {}
    bl = nblk * BF
    for nm in ("xt_t", "xt_i", "xt_a"):
        ins[nm] = nc.dram_tensor(nm, [D, bl], BF16, kind="ExternalInput").ap()
    ins["wqkv_p"] = nc.dram_tensor("wqkv_p", [128, NCH * NCH * 384], BF16, kind="ExternalInput").ap()
    for nm, w in (("wsa_p", 1024), ("wq_p", 1024), ("wkv_p", 2048), ("wca_p", 1024)):
        ins[nm] = nc.dram_tensor(nm, [128, NCH * w], BF16, kind="ExternalInput").ap()
    ins["sel"] = nc.dram_tensor("sel", [128, NCH * 16], BF16, kind="ExternalInput").ap()
    ins["selb"] = nc.dram_tensor("selb", [96, NCH * 128], BF16, kind="ExternalInput").ap()
    ins["ones_col"] = nc.dram_tensor("ones_col", [128, 1], BF16, kind="ExternalInput").ap()
    ins["ones_row"] = nc.dram_tensor("ones_row", [65, 128], BF16, kind="ExternalInput").ap()
    for nm in ("sab", "l1g", "l1b", "cab", "l2g", "l2b"):
        ins[nm] = nc.dram_tensor(nm, [128, NCH], F32, kind="ExternalInput").ap()
    outs = {"out_t": nc.dram_tensor("out_t", [D, bl], F32, kind="ExternalOutput").ap()}

    with tile.TileContext(nc) as tc:
        build(tc, outs, ins, nblk)
    nc.compile()
    _CACHE[nblk] = nc
    return nc


def kernel(c_text, c_image, c_audio, Wqkv, sa_proj_w, sa_proj_b, ln1_g, ln1_b,
           Wq, Wkv, ca_proj_w, ca_proj_b, ln2_g, ln2_b, _trace=False):
    bf = ml_dtypes.bfloat16
    shared = _prep_shared(Wqkv, sa_proj_w, sa_proj_b, ln1_g, ln1_b, Wq, Wkv,
                          ca_proj_w, ca_proj_b, ln2_g, ln2_b)
    xT = {
        "xt_t": np.ascontiguousarray(np.asarray(c_text, np.float32).T).astype(bf),
        "xt_i": np.ascontiguousarray(np.asarray(c_image, np.float32).T).astype(bf),
        "xt_a": np.ascontiguousarray(np.asarray(c_audio, np.float32).T).astype(bf),
    }
    in_maps = []
    for s in range(NCORES):
        sl = slice(s * BLOC, (s + 1) * BLOC)
        m = dict(shared)
        for k in xT:
            m[k] = np.ascontiguousarray(xT[k][:, sl])
        in_maps.append(m)
    nc = _get_program(NBLK_HW)
    res = bass_utils.run_bass_kernel_spmd(nc, in_maps, core_ids=list(range(NCORES)),
                                          trace=_trace)
    out = np.concatenate([np.asarray(r["out_t"]).T for r in res.results], axis=0)
    if _trace:
        kernel.last_results = res
    return out.astype(np.float32)
